# revision 67
# baseline (speedup 1.0000x reference)
"""DGCNN forward kernel for Trainium2 (8 NeuronCores, batch-parallel).

Strategy (per core = one sample of the batch), phase-major per layer to keep
engines streaming instead of the lockstep scores->lists->gather round-trips:
  - pre:    bf16 copies of x / weights, xx row, u = wa@x, v = (wb-wa)@x
            (bf16 matmuls, fp32 PSUM), u mean-centered.
  - phase1: for each of 16 row tiles: score matmul S = [x;1]^T[2x;-xx] (bf16,
            single PE op via augmentation when C<128), top-24 per row via
            3 rounds of DVE max8/find_index8/match_replace8, index lists
            transposed on PE into gather layout (group 0 only).
  - repl:   one batch of 14 SBUF DMAs replicates the index lists to the other
            16-partition groups (ap_gather wants per-core copies).
  - phase2: 64 ap_gathers (GPSIMD, runs ahead through a deep pool) + DVE
            segmented max/sum reduces + ACT square-accumulate for BN moments.
  - stats:  per-core BN moment terms, tiny AllReduce (syncBN), affine+lrelu.
  - final:  1x1 conv (bf16 PE) + BN + lrelu, stats via AllReduce.
"""

import numpy as np

B, C0, N = 8, 3, 2048
K = 20
EPS = 1e-5
LAYERS = [(3, 64), (64, 64), (64, 128), (128, 256)]  # (C_in, O)
NT = N // 128          # 16 row tiles
NCH = N // 512         # 4 matmul free-dim chunks
NCK = N // 64          # 32 gather chunks (64 points each)
CKR = 64               # points per gather chunk
NEG = -1.0e38

_CACHE = {}


def _build():
    import concourse.bass as bass
    import concourse.mybir as mybir
    from concourse import bacc
    from concourse.tile import TileContext

    dt = mybir.dt
    Alu = mybir.AluOpType
    Act = mybir.ActivationFunctionType

    nc = bacc.Bacc("TRN2", target_bir_lowering=False, debug=False,
                   enable_asserts=False, num_devices=8)

    # ---------------- DRAM I/O ----------------
    x_in = nc.dram_tensor("x0", [C0, N], dt.float32, kind="ExternalInput").ap()
    waT, wbmaT, gv, bv = {}, {}, {}, {}
    for li, (C, O) in enumerate(LAYERS):
        waT[li] = nc.dram_tensor(f"waT{li}", [C, O], dt.float32, kind="ExternalInput").ap()
        wbmaT[li] = nc.dram_tensor(f"wbmaT{li}", [C, O], dt.float32, kind="ExternalInput").ap()
        gv[li] = nc.dram_tensor(f"g{li}", [O, 1], dt.float32, kind="ExternalInput").ap()
        bv[li] = nc.dram_tensor(f"b{li}", [O, 1], dt.float32, kind="ExternalInput").ap()
    xdbg = [nc.dram_tensor(f"xdbg{i}", [128, N], dt.float32,
                           kind="ExternalOutput").ap() for i in range(4)]
    w5T_d = nc.dram_tensor("w5T", [512, 1024], dt.float32, kind="ExternalInput").ap()
    g5_d = nc.dram_tensor("g5", [1024, 1], dt.float32, kind="ExternalInput").ap()
    b5_d = nc.dram_tensor("b5", [1024, 1], dt.float32, kind="ExternalInput").ap()
    out_d = nc.dram_tensor("out", [1024, N], dt.float32, kind="ExternalOutput").ap()

    def sb(name, shape, dtype=dt.float32):
        return nc.alloc_sbuf_tensor(name, list(shape), dtype).ap()

    with TileContext(nc) as tc:
        # ---------------- persistent SBUF ----------------
        h = [sb("h0", [128, N]), sb("h1", [128, N]),
             sb("h2", [128, N]), sb("h3", [128, N])]
        ones_row = sb("ones_row", [1, N])
        nc.vector.memset(ones_row, 1.0)
        # identity for PE transpose: ident[p, f] = (f - p == 0)
        ident = sb("ident", [128, 128])
        iota_fp = sb("iota_fp", [128, 128], dt.int32)
        nc.gpsimd.iota(iota_fp, pattern=[[1, 128]], base=0, channel_multiplier=-1)
        nc.vector.tensor_scalar(out=ident, in0=iota_fp, scalar1=0, scalar2=None,
                                op0=Alu.is_equal)
        # replication selectors: REP1[c, o] = (c == o%16)  (16 nn slots),
        # REP2[c, o] = (c == 16 + (o%16)%4)  (slots 16-19 duplicated 4x)
        rep1 = sb("rep1", [24, 128])
        rep2 = sb("rep2", [24, 128])
        iota_r = sb("iota_r", [32, 128], dt.int32)
        nc.gpsimd.iota(iota_r, pattern=[[0, 8], [1, 16]], base=0,
                       channel_multiplier=-1)
        nc.vector.tensor_scalar(out=rep1, in0=iota_r[0:24, :], scalar1=0,
                                scalar2=None, op0=Alu.is_equal)
        nc.gpsimd.iota(iota_r, pattern=[[0, 32], [1, 4]], base=16,
                       channel_multiplier=-1)
        nc.vector.tensor_scalar(out=rep2, in0=iota_r[0:24, :], scalar1=0,
                                scalar2=None, op0=Alu.is_equal)

        w5T_sb = sb("w5T_sb", [128, 4 * 1024])   # 4 c-blocks side by side
        for cb in range(4):
            nc.sync.dma_start(w5T_sb[:, cb * 1024:(cb + 1) * 1024],
                              w5T_d[cb * 128:(cb + 1) * 128, :])

        # x2 needs its own base-partition-0 tensor (matmul operands share base)
        x2_sb = sb("x2_sb", [64, N])

        for li, (C, O) in enumerate(LAYERS):
            OT = (O + 127) // 128            # o-tiles
            ow = [min(128, O - ot * 128) for ot in range(OT)]
            AUG = C < 128
            CA = C + 1 if AUG else C
            CH = 128 if O >= 128 else 64     # gather channel count
            NG = CH // 16                    # 16-partition groups
            # 64-channel layers: replicate u into partitions 64-127 and give
            # the upper 4 Q7 cores the other half of the point stream, halving
            # the per-core gather index work (the layer bottleneck).
            SPLIT = CH == 64

            with tc.tile_pool(name=f"pp{li}", bufs=1) as pp, \
                 tc.tile_pool(name=f"ps{li}", bufs=4, space="PSUM") as psc, \
                 tc.tile_pool(name=f"pt{li}", bufs=4, space="PSUM") as psT2:

                def scope_in(s):
                    sid, _ = nc.enter_named_scope(f"L{li}_{s}", notify=False)
                    return (f"L{li}_{s}", sid)

                def scope_out(h_):
                    nc.leave_named_scope(h_[0], h_[1], notify=False)

                _sc = scope_in("pre")
                # ---- layer input (fp32, partitions 0..C-1) ----
                if li == 0:
                    xc = pp.tile([C0, N], dt.float32, tag="x0_sb")
                    nc.sync.dma_start(xc, x_in)
                elif li == 1:
                    xc = h[0][0:64, :]
                elif li == 2:
                    xc = x2_sb[:, :]
                else:
                    xc = h[1][:, :]

                # ---- score operands (fp32: bf16 is too coarse for the kNN
                # threshold region once |S| ~ C) ----
                ones_col = pp.tile([C, 1], dt.float32, tag="ones_col")
                nc.vector.memset(ones_col, 1.0)
                b2x = pp.tile([CA, N], dt.float32, tag="b2x")
                nc.scalar.mul(b2x[0:C, :], xc, 2.0)
                # xsq scratch lives in `up` (not yet written at this point)
                up = pp.tile([128, OT * N], dt.float32, tag="up")
                xsq = up[0:C, 0:N]
                nc.vector.tensor_tensor(out=xsq, in0=xc, in1=xc, op=Alu.mult)
                bnxx = pp.tile([1, N], dt.float32, tag="bnxx")
                for nch in range(NCH):
                    xxp = psc.tile([128, 512], dt.float32, tag="pt512")
                    nc.tensor.matmul(xxp[0:1, :], lhsT=ones_col,
                                     rhs=xsq[:, nch * 512:(nch + 1) * 512],
                                     start=True, stop=True)
                    nc.scalar.mul(bnxx[:, nch * 512:(nch + 1) * 512], xxp[0:1, :], -1.0)
                if AUG:
                    xaug = pp.tile([CA, N], dt.float32, tag="xaug")
                    nc.vector.tensor_copy(xaug[0:C, :], xc)
                    # rows at arbitrary partition base: fill via DMA
                    nc.sync.dma_start(xaug[C:C + 1, :], ones_row)
                    nc.sync.dma_start(b2x[C:C + 1, :], bnxx)

                # ---- u, v matmuls (fp32), centering ----
                waT_sb = pp.tile([C, O], dt.float32, tag="waT_sb")
                nc.sync.dma_start(waT_sb, waT[li])
                wbmaT_sb = pp.tile([C, O], dt.float32, tag="wbmaT_sb")
                nc.sync.dma_start(wbmaT_sb, wbmaT[li])

                vv = pp.tile([128, OT * N], dt.float32, tag="vv")
                sm = pp.tile([128, 96], dt.float32, tag="sm")
                neg_mu = sm[:, 0:2]
                sum_v = sm[:, 2:4]
                sum_u = sm[:, 4:6]
                sum_q = sm[:, 6:8]
                sum_s = sm[:, 8:10]
                svs = sm[:, 10:12]
                sv2 = sm[:, 12:14]
                vbar = sm[:, 14:16]
                beta = sm[:, 16:18]
                t1a = sm[:, 18:20]
                S1p = sm[:, 20:22]
                tA = sm[:, 22:24]
                tBt = sm[:, 24:26]
                S2p = sm[:, 26:28]
                tC = sm[:, 28:30]
                tD = sm[:, 30:32]
                mean = sm[:, 32:34]
                e2 = sm[:, 34:36]
                varp = sm[:, 36:38]
                rec = sm[:, 38:40]
                rsq = sm[:, 40:42]
                aco = sm[:, 42:44]
                bi = sm[:, 44:46]
                gsb = sm[:, 46:48]
                bsb = sm[:, 48:50]
                ar_in = sm[:, 50:54]
                ar_out = sm[:, 54:58]
                upart = sm[:, 58:58 + 2 * NCH]     # per-chunk accum partials
                vpart = sm[:, 58 + 2 * NCH:58 + 4 * NCH]

                for ot in range(OT):
                    w_ = ow[ot]
                    for nch in range(NCH):
                        upp = psc.tile([128, 512], dt.float32, tag="pt512")
                        nc.tensor.matmul(upp[0:w_, :],
                                         lhsT=waT_sb[:, ot * 128:ot * 128 + w_],
                                         rhs=xc[:, nch * 512:(nch + 1) * 512],
                                         start=True, stop=True)
                        nc.scalar.activation(
                            up[0:w_, ot * N + nch * 512: ot * N + (nch + 1) * 512],
                            upp[0:w_, :], Act.Copy)
                        vpp = psc.tile([128, 512], dt.float32, tag="pt512")
                        nc.tensor.matmul(vpp[0:w_, :],
                                         lhsT=wbmaT_sb[:, ot * 128:ot * 128 + w_],
                                         rhs=xc[:, nch * 512:(nch + 1) * 512],
                                         start=True, stop=True)
                        nc.scalar.activation(
                            vv[0:w_, ot * N + nch * 512: ot * N + (nch + 1) * 512],
                            vpp[0:w_, :], Act.Copy)
                    nc.vector.tensor_reduce(out=sum_u[0:w_, ot:ot + 1],
                                            in_=up[0:w_, ot * N:(ot + 1) * N],
                                            op=Alu.add, axis=mybir.AxisListType.X)
                    nc.vector.tensor_reduce(out=sum_v[0:w_, ot:ot + 1],
                                            in_=vv[0:w_, ot * N:(ot + 1) * N],
                                            op=Alu.add, axis=mybir.AxisListType.X)
                    nc.scalar.mul(neg_mu[0:w_, ot:ot + 1], sum_u[0:w_, ot:ot + 1],
                                  -1.0 / N)
                    nc.scalar.activation(up[0:w_, ot * N:(ot + 1) * N],
                                         up[0:w_, ot * N:(ot + 1) * N], Act.Identity,
                                         bias=neg_mu[0:w_, ot:ot + 1], scale=1.0)

                if SPLIT:
                    # replicate u so the upper 4 cores can gather the other
                    # half of the point stream in parallel (emitted before the
                    # list-replication DMAs so the sync queue can issue it
                    # before the first gathers need it)
                    nc.sync.dma_start(up[64:128, 0:N], up[0:64, 0:N])
                scope_out(_sc)
                _sc = scope_in("topk")
                # ---- phase 1: scores + topk + index lists ----
                # lg1[p, n] = If[n, p%16] (16 nn slots) is built replicated
                # across the 16-partition groups by one selector matmul.
                # lg2 holds slots 16-19 in the baseline 8-slot parity layout
                # (4 junk dups): even points in partitions 0-7, odd in 8-15
                # (staged via DMA), replicated per layer.
                # SPLIT layers: partitions 0-63 list the row tile's first 64
                # points, partitions 64-127 the other 64 (staged + DMA'd).
                PR = 64 if SPLIT else 128        # points listed per rt per half
                lg1 = pp.tile([128 if SPLIT else CH, NT * PR], dt.int16, tag="lg1")
                lg2 = pp.tile([128 if SPLIT else CH, NT * PR // 2], dt.int16, tag="lg2")
                st8odd = pp.tile([8, NT * PR // 2], dt.int16, tag="st8odd")
                if SPLIT:
                    stB1 = pp.tile([64, NT * 64], dt.int16, tag="stB1")
                    stBe = pp.tile([8, NT * 32], dt.int16, tag="stBe")
                    stBo = pp.tile([8, NT * 32], dt.int16, tag="stBo")
                tps = {}
                with tc.tile_pool(name=f"paS{li}", bufs=(3 if li < 3 else 2)) as paS, \
                     tc.tile_pool(name=f"paI{li}", bufs=3) as paI:
                    def emit_scores_topk(rt):
                        Ssb = paS.tile([128, N], dt.float32, tag="Ssb")
                        for nch in range(NCH):
                            Spc = psc.tile([128, 512], dt.float32, tag="pt512")
                            if AUG:
                                nc.tensor.matmul(
                                    Spc, lhsT=xaug[:, rt * 128:(rt + 1) * 128],
                                    rhs=b2x[:, nch * 512:(nch + 1) * 512],
                                    start=True, stop=True)
                            else:
                                nc.tensor.matmul(
                                    Spc, lhsT=xc[:, rt * 128:(rt + 1) * 128],
                                    rhs=b2x[0:C, nch * 512:(nch + 1) * 512],
                                    start=True, stop=False)
                                nc.tensor.matmul(
                                    Spc, lhsT=ones_row[:, rt * 128:(rt + 1) * 128],
                                    rhs=bnxx[:, nch * 512:(nch + 1) * 512],
                                    start=False, stop=True)
                            nc.scalar.activation(Ssb[:, nch * 512:(nch + 1) * 512],
                                                 Spc, Act.Copy)
                        V = paI.tile([128, 24], dt.float32, tag="V")
                        I = paI.tile([128, 24], dt.uint16, tag="I")
                        Sw = paS.tile([128, N], dt.float32, tag="Ssb")
                        nc.vector.max(out=V[:, 0:8], in_=Ssb)
                        nc.vector.max_index(out=I[:, 0:8], in_max=V[:, 0:8],
                                            in_values=Ssb)
                        nc.vector.match_replace(out=Sw, in_to_replace=V[:, 0:8],
                                                in_values=Ssb, imm_value=NEG)
                        nc.vector.max(out=V[:, 8:16], in_=Sw)
                        nc.vector.max_index(out=I[:, 8:16], in_max=V[:, 8:16],
                                            in_values=Sw)
                        nc.vector.match_replace(out=Sw, in_to_replace=V[:, 8:16],
                                                in_values=Sw, imm_value=NEG)
                        nc.vector.max(out=V[:, 16:24], in_=Sw)
                        nc.vector.max_index(out=I[:, 16:24], in_max=V[:, 16:24],
                                            in_values=Sw)
                        If = paI.tile([128, 24], dt.float32, tag="If")
                        nc.vector.tensor_copy(If, I)
                        # transpose once, then replicate across 16-partition
                        # groups with constant selector matmuls (no DMAs)
                        t24 = psT2.tile([128, 128], dt.float32, tag="ptT")
                        nc.tensor.transpose(t24[0:24, :], If, ident)
                        IfT = paI.tile([24, 128], dt.float32, tag="IfT")
                        nc.vector.tensor_copy(IfT, t24[0:24, :])
                        if SPLIT:
                            tpa = psT2.tile([128, 128], dt.float32, tag="ptT")
                            nc.tensor.matmul(tpa[0:64, 0:64], lhsT=rep1[:, 0:64],
                                             rhs=IfT[:, 0:64], start=True, stop=True)
                            nc.tensor.matmul(tpa[0:64, 64:128], lhsT=rep1[:, 0:64],
                                             rhs=IfT[:, 64:128], start=True, stop=True)
                        else:
                            tpa = psT2.tile([128, 128], dt.float32, tag="ptT")
                            nc.tensor.matmul(tpa[0:CH, :], lhsT=rep1[:, 0:CH],
                                             rhs=IfT, start=True, stop=True)
                        tpb = psT2.tile([128, 128], dt.float32, tag="ptT")
                        nc.tensor.transpose(tpb[0:8, :], If[:, 16:24], ident)
                        return tpa, tpb

                    def emit_lists(rt, tpa, tpb):
                        if SPLIT:
                            nc.vector.tensor_copy(lg1[0:64, rt * 64:(rt + 1) * 64],
                                                  tpa[0:64, 0:64])
                            nc.vector.tensor_copy(stB1[:, rt * 64:(rt + 1) * 64],
                                                  tpa[0:64, 64:128])
                            nc.vector.tensor_copy(lg2[0:8, rt * 32:(rt + 1) * 32],
                                                  tpb[0:8, 0:64:2])
                            nc.vector.tensor_copy(st8odd[:, rt * 32:(rt + 1) * 32],
                                                  tpb[0:8, 1:64:2])
                            nc.vector.tensor_copy(stBe[:, rt * 32:(rt + 1) * 32],
                                                  tpb[0:8, 64:128:2])
                            nc.vector.tensor_copy(stBo[:, rt * 32:(rt + 1) * 32],
                                                  tpb[0:8, 65:128:2])
                            return
                        nc.vector.tensor_copy(lg1[:, rt * 128:(rt + 1) * 128],
                                              tpa[0:CH, :])
                        # baseline 8-slot parity layout for slots 16-19
                        nc.vector.tensor_copy(lg2[0:8, rt * 64:(rt + 1) * 64],
                                              tpb[0:8, 0::2])
                        nc.vector.tensor_copy(st8odd[:, rt * 64:(rt + 1) * 64],
                                              tpb[0:8, 1::2])

                    def repl_lists(hh):
                        # replicate/stage the lists for row tiles of half hh:
                        # emitting the first batch mid-topk lets the gathers
                        # start while the DVE is still on top-k of tiles 8-15
                        c1 = slice(hh * 8 * PR, (hh + 1) * 8 * PR)
                        c2 = slice(hh * 8 * (PR // 2), (hh + 1) * 8 * (PR // 2))
                        nc.sync.dma_start(lg2[8:16, c2], st8odd[:, c2])
                        if SPLIT:
                            nc.sync.dma_start(lg1[64:128, c1], stB1[:, c1])
                            nc.sync.dma_start(lg2[64:72, c2], stBe[:, c2])
                            nc.sync.dma_start(lg2[72:80, c2], stBo[:, c2])
                            for g in (1, 2, 3):
                                nc.sync.dma_start(lg2[g * 16:(g + 1) * 16, c2],
                                                  lg2[0:16, c2])
                                nc.sync.dma_start(
                                    lg2[64 + g * 16:64 + (g + 1) * 16, c2],
                                    lg2[64:80, c2])
                        else:
                            for g in range(1, NG):
                                nc.sync.dma_start(lg2[g * 16:(g + 1) * 16, c2],
                                                  lg2[0:16, c2])

                    for it in range(NT + 1):
                        if it - 1 >= 0:
                            emit_lists(it - 1, *tps.pop(it - 1))
                        if it == NT // 2:
                            repl_lists(0)
                        if it < NT:
                            tps[it] = emit_scores_topk(it)
                    repl_lists(1)

                scope_out(_sc)
                _sc = scope_in("gath")
                # ---- phase 2: gathers + reduces + BN moment accum ----
                # one g1 (16 slots) + one g2 (8-slot parity, 4 valid) gather
                # per row tile: gather cost scales with index count, so big
                # batches amortize the Q7 per-request overhead.
                mm = pp.tile([128, OT * N], dt.float32, tag="mm")
                s_sb = pp.tile([128, OT * N], dt.float32, tag="s_sb")
                qac = pp.tile([128, OT * 32], dt.float32, tag="qac")
                nc.vector.memset(qac, 0.0)
                sqj = pp.tile([128, 2048], dt.float32, tag="sqj")
                if SPLIT:
                    msplit = pp.tile([128, NT * 64], dt.float32, tag="msplit")
                    ssplit = pp.tile([128, NT * 64], dt.float32, tag="ssplit")
                with tc.tile_pool(name=f"pg{li}", bufs=2) as pg, \
                     tc.tile_pool(name=f"pgs{li}", bufs=4) as pgs:
                  if SPLIT:
                    w_ = 64
                    # 2048-idx instructions are the ap_gather sweet spot:
                    # 16-slot lists per 2 row tiles, parity lists per 4
                    for r4 in range(NT // 4):
                        gB = pg.tile([128, 2048], dt.float32, tag="g2")
                        nc.gpsimd.ap_gather(
                            gB, up[:, 0:N], lg2[:, r4 * 128:(r4 + 1) * 128],
                            channels=128, num_elems=N, d=1, num_idxs=2048)
                        bsl = slice(r4 * 256, (r4 + 1) * 256)
                        m2 = pgs.tile([128, 256], dt.float32, tag="m2")
                        nc.vector.tensor_reduce(
                            out=m2,
                            in_=gB.rearrange("p (n k) -> p n k", k=8)[:, :, 0:4],
                            op=Alu.max, axis=mybir.AxisListType.X)
                        s2 = pgs.tile([128, 256], dt.float32, tag="s2")
                        nc.vector.tensor_reduce(
                            out=s2,
                            in_=gB.rearrange("p (n k) -> p n k", k=8)[:, :, 0:4],
                            op=Alu.add, axis=mybir.AxisListType.X)
                        g2v = gB.rearrange("p (n k) -> p n k", k=8)[:, :, 0:4]
                        sqv = sqj[:, 0:1024].rearrange("p (n k) -> p n k", k=4)
                        nc.scalar.activation(
                            sqv, g2v, Act.Square,
                            accum_out=qac[:, 8 + r4:9 + r4])
                        for h2 in range(2):
                            r2 = 2 * r4 + h2
                            gA = pg.tile([128, 2048], dt.float32, tag="g1")
                            nc.gpsimd.ap_gather(
                                gA, up[:, 0:N], lg1[:, r2 * 128:(r2 + 1) * 128],
                                channels=128, num_elems=N, d=1, num_idxs=2048)
                            msl = slice(r2 * 128, (r2 + 1) * 128)
                            nc.vector.tensor_reduce(
                                out=msplit[:, msl],
                                in_=gA.rearrange("p (n k) -> p n k", k=16),
                                op=Alu.max, axis=mybir.AxisListType.X)
                            nc.vector.tensor_tensor(
                                out=msplit[:, msl], in0=msplit[:, msl],
                                in1=m2[:, h2 * 128:(h2 + 1) * 128], op=Alu.max)
                            s1 = pgs.tile([128, 128], dt.float32, tag="s1")
                            nc.vector.tensor_reduce(
                                out=s1,
                                in_=gA.rearrange("p (n k) -> p n k", k=16),
                                op=Alu.add, axis=mybir.AxisListType.X)
                            nc.vector.tensor_tensor(
                                out=ssplit[:, msl], in0=s1,
                                in1=s2[:, h2 * 128:(h2 + 1) * 128], op=Alu.add)
                            nc.scalar.activation(
                                sqj, gA, Act.Square,
                                accum_out=qac[:, r2:r2 + 1])
                  else:
                    # L3 measured best with per-row-tile parity gathers; L2
                    # with per-2-tile (2048-idx) ones
                    RG = 1 if li == 3 else 2
                    for ot in range(OT):
                        w_ = ow[ot]
                        wch = ((w_ + 15) // 16) * 16
                        usrc = up[0:wch, ot * N:(ot + 1) * N]
                        for rg in range(NT // RG):
                            g2 = pg.tile([CH, 1024 * RG], dt.float32, tag="g2")
                            nc.gpsimd.ap_gather(
                                g2[0:wch, :], usrc,
                                lg2[0:wch, rg * 64 * RG: (rg + 1) * 64 * RG],
                                channels=wch, num_elems=N, d=1, num_idxs=1024 * RG)
                            m2 = pgs.tile([128, 128 * RG], dt.float32, tag="m2")
                            nc.vector.tensor_reduce(
                                out=m2[0:w_, :],
                                in_=g2[0:w_, :].rearrange("p (n k) -> p n k", k=8)[:, :, 0:4],
                                op=Alu.max, axis=mybir.AxisListType.X)
                            s2 = pgs.tile([128, 128 * RG], dt.float32, tag="s2")
                            nc.vector.tensor_reduce(
                                out=s2[0:w_, :],
                                in_=g2[0:w_, :].rearrange("p (n k) -> p n k", k=8)[:, :, 0:4],
                                op=Alu.add, axis=mybir.AxisListType.X)
                            g2v = g2[0:w_, :].rearrange("p (n k) -> p n k", k=8)[:, :, 0:4]
                            sqv = sqj[0:w_, 0:512 * RG].rearrange("p (n k) -> p n k", k=4)
                            nc.scalar.activation(
                                sqv, g2v, Act.Square,
                                accum_out=qac[0:w_, ot * 32 + 16 + rg: ot * 32 + 17 + rg])
                            for h2 in range(RG):
                                rt = RG * rg + h2
                                msl = slice(ot * N + rt * 128, ot * N + (rt + 1) * 128)
                                g1 = pg.tile([CH, 2048], dt.float32, tag="g1")
                                nc.gpsimd.ap_gather(
                                    g1[0:wch, :], usrc,
                                    lg1[0:wch, rt * 128: (rt + 1) * 128],
                                    channels=wch, num_elems=N, d=1, num_idxs=2048)
                                nc.vector.tensor_reduce(
                                    out=mm[0:w_, msl],
                                    in_=g1[0:w_, :].rearrange("p (n k) -> p n k", k=16),
                                    op=Alu.max, axis=mybir.AxisListType.X)
                                nc.vector.tensor_tensor(
                                    out=mm[0:w_, msl], in0=mm[0:w_, msl],
                                    in1=m2[0:w_, h2 * 128:(h2 + 1) * 128], op=Alu.max)
                                s1 = pgs.tile([128, 128], dt.float32, tag="s1")
                                nc.vector.tensor_reduce(
                                    out=s1[0:w_, :],
                                    in_=g1[0:w_, :].rearrange("p (n k) -> p n k", k=16),
                                    op=Alu.add, axis=mybir.AxisListType.X)
                                nc.vector.tensor_tensor(
                                    out=s_sb[0:w_, msl], in0=s1[0:w_, :],
                                    in1=s2[0:w_, h2 * 128:(h2 + 1) * 128], op=Alu.add)
                                # square to a scratch tile: in-place would
                                # WAR-couple the gathers to the scalar engine
                                nc.scalar.activation(
                                    sqj[0:w_, :], g1[0:w_, :], Act.Square,
                                    accum_out=qac[0:w_, ot * 32 + rt: ot * 32 + rt + 1])
                if SPLIT:
                    # fold the upper-half results (points 64-127 of each row
                    # tile, computed at partitions 64-127) back down
                    nc.sync.dma_start(
                        mm[0:64, :].rearrange("p (r n) -> p r n", n=128)[:, :, 0:64],
                        msplit[0:64, :].rearrange("p (r n) -> p r n", n=64))
                    nc.sync.dma_start(
                        mm[0:64, :].rearrange("p (r n) -> p r n", n=128)[:, :, 64:128],
                        msplit[64:128, :].rearrange("p (r n) -> p r n", n=64))
                    nc.sync.dma_start(
                        s_sb[0:64, :].rearrange("p (r n) -> p r n", n=128)[:, :, 0:64],
                        ssplit[0:64, :].rearrange("p (r n) -> p r n", n=64))
                    nc.sync.dma_start(
                        s_sb[0:64, :].rearrange("p (r n) -> p r n", n=128)[:, :, 64:128],
                        ssplit[64:128, :].rearrange("p (r n) -> p r n", n=64))
                    nc.sync.dma_start(qac[0:64, 12:24], qac[64:128, 0:12])
                for ot in range(OT):
                    w_ = ow[ot]
                    nc.vector.tensor_reduce(out=sum_q[0:w_, ot:ot + 1],
                                            in_=qac[0:w_, ot * 32:(ot + 1) * 32],
                                            op=Alu.add, axis=mybir.AxisListType.X)

                scope_out(_sc)
                _sc = scope_in("stat")
                # ---- per-core stat terms + AllReduce ----
                for ot in range(OT):
                    w_ = ow[ot]
                    ssl = s_sb[0:w_, ot * N:(ot + 1) * N]
                    vsl = vv[0:w_, ot * N:(ot + 1) * N]
                    nc.vector.tensor_reduce(out=sum_s[0:w_, ot:ot + 1], in_=ssl,
                                            op=Alu.add, axis=mybir.AxisListType.X)
                    # up is fully consumed by the gathers at this point; reuse
                    # its slice as the accumulate-op scratch destination.
                    junk = up[:, ot * N:(ot + 1) * N]
                    nc.vector.scalar_tensor_tensor(out=junk[0:w_, :], in0=ssl, scalar=1.0,
                                                   in1=vsl, op0=Alu.mult, op1=Alu.mult,
                                                   accum_out=svs[0:w_, ot:ot + 1])
                    nc.vector.scalar_tensor_tensor(out=junk[0:w_, :], in0=vsl, scalar=1.0,
                                                   in1=vsl, op0=Alu.mult, op1=Alu.mult,
                                                   accum_out=sv2[0:w_, ot:ot + 1])
                    nc.scalar.mul(vbar[0:w_, ot:ot + 1], sum_v[0:w_, ot:ot + 1], 1.0 / N)
                    nc.vector.scalar_tensor_tensor(out=beta[0:w_, ot:ot + 1],
                                                   in0=neg_mu[0:w_, ot:ot + 1], scalar=-1.0,
                                                   in1=vbar[0:w_, ot:ot + 1],
                                                   op0=Alu.mult, op1=Alu.add)
                    nc.vector.scalar_tensor_tensor(out=t1a[0:w_, ot:ot + 1],
                                                   in0=vbar[0:w_, ot:ot + 1], scalar=-float(N),
                                                   in1=sum_v[0:w_, ot:ot + 1],
                                                   op0=Alu.mult, op1=Alu.add)
                    nc.vector.scalar_tensor_tensor(out=S1p[0:w_, ot:ot + 1],
                                                   in0=t1a[0:w_, ot:ot + 1], scalar=float(K),
                                                   in1=sum_s[0:w_, ot:ot + 1],
                                                   op0=Alu.mult, op1=Alu.add)
                    nc.vector.tensor_tensor(out=tA[0:w_, ot:ot + 1], in0=vbar[0:w_, ot:ot + 1],
                                            in1=sum_s[0:w_, ot:ot + 1], op=Alu.mult)
                    nc.vector.scalar_tensor_tensor(out=tA[0:w_, ot:ot + 1],
                                                   in0=tA[0:w_, ot:ot + 1], scalar=-1.0,
                                                   in1=svs[0:w_, ot:ot + 1],
                                                   op0=Alu.mult, op1=Alu.add)
                    nc.vector.tensor_tensor(out=tBt[0:w_, ot:ot + 1], in0=vbar[0:w_, ot:ot + 1],
                                            in1=vbar[0:w_, ot:ot + 1], op=Alu.mult)
                    nc.vector.scalar_tensor_tensor(out=tBt[0:w_, ot:ot + 1],
                                                   in0=tBt[0:w_, ot:ot + 1], scalar=-float(N),
                                                   in1=sv2[0:w_, ot:ot + 1],
                                                   op0=Alu.mult, op1=Alu.add)
                    nc.vector.scalar_tensor_tensor(out=S2p[0:w_, ot:ot + 1],
                                                   in0=tA[0:w_, ot:ot + 1], scalar=2.0,
                                                   in1=sum_q[0:w_, ot:ot + 1],
                                                   op0=Alu.mult, op1=Alu.add)
                    nc.vector.scalar_tensor_tensor(out=S2p[0:w_, ot:ot + 1],
                                                   in0=tBt[0:w_, ot:ot + 1], scalar=float(K),
                                                   in1=S2p[0:w_, ot:ot + 1],
                                                   op0=Alu.mult, op1=Alu.add)
                    cntl = float(N * K)
                    nc.vector.scalar_tensor_tensor(out=ar_in[0:w_, 2 * ot:2 * ot + 1],
                                                   in0=beta[0:w_, ot:ot + 1], scalar=cntl,
                                                   in1=S1p[0:w_, ot:ot + 1],
                                                   op0=Alu.mult, op1=Alu.add)
                    nc.vector.tensor_tensor(out=tC[0:w_, ot:ot + 1], in0=beta[0:w_, ot:ot + 1],
                                            in1=S1p[0:w_, ot:ot + 1], op=Alu.mult)
                    nc.vector.scalar_tensor_tensor(out=tC[0:w_, ot:ot + 1],
                                                   in0=tC[0:w_, ot:ot + 1], scalar=2.0,
                                                   in1=S2p[0:w_, ot:ot + 1],
                                                   op0=Alu.mult, op1=Alu.add)
                    nc.vector.tensor_tensor(out=tD[0:w_, ot:ot + 1], in0=beta[0:w_, ot:ot + 1],
                                            in1=beta[0:w_, ot:ot + 1], op=Alu.mult)
                    nc.vector.scalar_tensor_tensor(out=ar_in[0:w_, 2 * ot + 1:2 * ot + 2],
                                                   in0=tD[0:w_, ot:ot + 1], scalar=cntl,
                                                   in1=tC[0:w_, ot:ot + 1],
                                                   op0=Alu.mult, op1=Alu.add)

                with tc.tile_pool(name=f"dr{li}", bufs=1, space="DRAM") as dram:
                    ari = dram.tile([128, 2 * OT], dt.float32)
                    aro = dram.tile([128, 2 * OT], dt.float32)
                    nc.sync.dma_start(ari[:], ar_in[:, 0:2 * OT])
                    nc.gpsimd.collective_compute(
                        "AllReduce", Alu.add, replica_groups=[list(range(8))],
                        ins=[ari.opt()], outs=[aro.opt()])
                    nc.sync.dma_start(ar_out[:, 0:2 * OT], aro[:])
                # overlap with the AllReduce: mm <- m' + v (no AR dependency)
                for ot in range(OT):
                    w_ = ow[ot]
                    nc.vector.tensor_tensor(out=mm[0:w_, ot * N:(ot + 1) * N],
                                            in0=mm[0:w_, ot * N:(ot + 1) * N],
                                            in1=vv[0:w_, ot * N:(ot + 1) * N], op=Alu.add)

                # post-AR: mean/var/scale/bias + activation
                nc.sync.dma_start(gsb[0:ow[0], 0:1], gv[li][0:ow[0], :])
                nc.sync.dma_start(bsb[0:ow[0], 0:1], bv[li][0:ow[0], :])
                if OT > 1:
                    nc.sync.dma_start(gsb[0:ow[1], 1:2], gv[li][128:128 + ow[1], :])
                    nc.sync.dma_start(bsb[0:ow[1], 1:2], bv[li][128:128 + ow[1], :])
                cntg = float(B * N * K)
                for ot in range(OT):
                    w_ = ow[ot]
                    nc.scalar.mul(mean[0:w_, ot:ot + 1], ar_out[0:w_, 2 * ot:2 * ot + 1], 1.0 / cntg)
                    nc.scalar.mul(e2[0:w_, ot:ot + 1], ar_out[0:w_, 2 * ot + 1:2 * ot + 2], 1.0 / cntg)
                    nc.vector.tensor_tensor(out=varp[0:w_, ot:ot + 1], in0=mean[0:w_, ot:ot + 1],
                                            in1=mean[0:w_, ot:ot + 1], op=Alu.mult)
                    nc.vector.scalar_tensor_tensor(out=varp[0:w_, ot:ot + 1],
                                                   in0=varp[0:w_, ot:ot + 1], scalar=-1.0,
                                                   in1=e2[0:w_, ot:ot + 1],
                                                   op0=Alu.mult, op1=Alu.add)
                    nc.vector.tensor_scalar_add(varp[0:w_, ot:ot + 1], varp[0:w_, ot:ot + 1], EPS)
                    nc.vector.reciprocal(rec[0:w_, ot:ot + 1], varp[0:w_, ot:ot + 1])
                    nc.scalar.sqrt(rsq[0:w_, ot:ot + 1], rec[0:w_, ot:ot + 1])
                    nc.vector.tensor_tensor(out=aco[0:w_, ot:ot + 1], in0=gsb[0:w_, ot:ot + 1],
                                            in1=rsq[0:w_, ot:ot + 1], op=Alu.mult)
                    nc.vector.tensor_tensor(out=bi[0:w_, ot:ot + 1], in0=neg_mu[0:w_, ot:ot + 1],
                                            in1=mean[0:w_, ot:ot + 1], op=Alu.add)
                    nc.vector.tensor_tensor(out=bi[0:w_, ot:ot + 1], in0=bi[0:w_, ot:ot + 1],
                                            in1=aco[0:w_, ot:ot + 1], op=Alu.mult)
                    nc.vector.scalar_tensor_tensor(out=bi[0:w_, ot:ot + 1],
                                                   in0=bi[0:w_, ot:ot + 1], scalar=-1.0,
                                                   in1=bsb[0:w_, ot:ot + 1],
                                                   op0=Alu.mult, op1=Alu.add)
                    if li == 0:
                        dst = h[0][0:64, :]
                    elif li == 1:
                        dst = x2_sb[:, :]
                    elif li == 2:
                        dst = h[1][:, :]
                    else:
                        dst = h[2][:, :] if ot == 0 else h[3][:, :]
                    zt = up[:, ot * N:(ot + 1) * N]
                    nc.scalar.activation(zt[0:w_, :], mm[0:w_, ot * N:(ot + 1) * N],
                                         Act.Identity,
                                         bias=bi[0:w_, ot:ot + 1], scale=aco[0:w_, ot:ot + 1])
                    nc.vector.scalar_tensor_tensor(out=dst[0:w_, :],
                                                   in0=zt[0:w_, :], scalar=0.2,
                                                   in1=zt[0:w_, :], op0=Alu.mult, op1=Alu.max)
                    if li == 1:
                        nc.sync.dma_start(h[0][64:128, :], x2_sb)
                scope_out(_sc)

        # ---------------- final conv + BN + lrelu ----------------
        with tc.tile_pool(name="pf", bufs=1) as pf, \
             tc.tile_pool(name="pfp", bufs=2, space="PSUM") as pfp:
            y_sb = []
            for ob in range(8):
                ytile = pf.tile([128, N], dt.float32, tag=f"y{ob}")
                y_sb.append(ytile)
            sm5 = pf.tile([128, 160], dt.float32, tag="sm5")
            sum_y = sm5[:, 0:8]
            mu5 = sm5[:, 8:16]
            nmu5 = sm5[:, 16:24]
            syc2 = sm5[:, 24:32]
            tE = sm5[:, 32:40]
            tF = sm5[:, 40:48]
            g5_sb = sm5[:, 48:56]
            b5_sb = sm5[:, 56:64]
            mean5 = sm5[:, 64:72]
            e25 = sm5[:, 72:80]
            var5 = sm5[:, 80:88]
            rec5 = sm5[:, 88:96]
            rsq5 = sm5[:, 96:104]
            a5 = sm5[:, 104:112]
            c5 = sm5[:, 112:120]
            ar5_in = sm5[:, 120:136]
            ar5_out = sm5[:, 136:152]
            w5b = pf.tile([128, 4 * 1024], dt.bfloat16, tag="w5b")
            nc.vector.tensor_copy(w5b, w5T_sb)
            hb = []
            for cb in range(4):
                hbt = pf.tile([128, N], dt.bfloat16, tag=f"hb{cb}")
                nc.vector.tensor_copy(hbt, h[cb])
                hb.append(hbt)
            for ob in range(8):
                for nch in range(NCH):
                    yp = pfp.tile([128, 512], dt.float32, tag="yp")
                    for cb in range(4):
                        nc.tensor.matmul(yp, lhsT=w5b[:, cb * 1024 + ob * 128:
                                                      cb * 1024 + (ob + 1) * 128],
                                         rhs=hb[cb][:, nch * 512:(nch + 1) * 512],
                                         start=(cb == 0), stop=(cb == 3))
                    nc.scalar.activation(y_sb[ob][:, nch * 512:(nch + 1) * 512], yp, Act.Copy)
                nc.vector.tensor_reduce(out=sum_y[:, ob:ob + 1], in_=y_sb[ob],
                                        op=Alu.add, axis=mybir.AxisListType.X)
                nc.scalar.mul(mu5[:, ob:ob + 1], sum_y[:, ob:ob + 1], 1.0 / N)
                nc.scalar.mul(nmu5[:, ob:ob + 1], sum_y[:, ob:ob + 1], -1.0 / N)
                yc = pf.tile([128, N], dt.float32, tag="yc")
                nc.scalar.activation(yc, y_sb[ob], Act.Identity,
                                     bias=nmu5[:, ob:ob + 1], scale=1.0)
                junk5 = pf.tile([128, N], dt.float32, tag="junk5")
                nc.vector.scalar_tensor_tensor(out=junk5, in0=yc, scalar=1.0, in1=yc,
                                               op0=Alu.mult, op1=Alu.mult,
                                               accum_out=syc2[:, ob:ob + 1])
                nc.vector.tensor_copy(ar5_in[:, 2 * ob:2 * ob + 1], sum_y[:, ob:ob + 1])
                nc.vector.tensor_tensor(out=tE[:, ob:ob + 1], in0=mu5[:, ob:ob + 1],
                                        in1=sum_y[:, ob:ob + 1], op=Alu.mult)
                nc.vector.scalar_tensor_tensor(out=tE[:, ob:ob + 1], in0=tE[:, ob:ob + 1],
                                               scalar=2.0, in1=syc2[:, ob:ob + 1],
                                               op0=Alu.mult, op1=Alu.add)
                nc.vector.tensor_tensor(out=tF[:, ob:ob + 1], in0=mu5[:, ob:ob + 1],
                                        in1=mu5[:, ob:ob + 1], op=Alu.mult)
                nc.vector.scalar_tensor_tensor(out=ar5_in[:, 2 * ob + 1:2 * ob + 2],
                                               in0=tF[:, ob:ob + 1], scalar=-float(N),
                                               in1=tE[:, ob:ob + 1],
                                               op0=Alu.mult, op1=Alu.add)
            with tc.tile_pool(name="dr5", bufs=1, space="DRAM") as dram5:
                ari5 = dram5.tile([128, 16], dt.float32)
                aro5 = dram5.tile([128, 16], dt.float32)
                nc.sync.dma_start(ari5[:], ar5_in)
                nc.gpsimd.collective_compute(
                    "AllReduce", Alu.add, replica_groups=[list(range(8))],
                    ins=[ari5.opt()], outs=[aro5.opt()])
                nc.sync.dma_start(ar5_out, aro5[:])
            for ob in range(8):
                nc.sync.dma_start(g5_sb[:, ob:ob + 1], g5_d[ob * 128:(ob + 1) * 128, :])
                nc.sync.dma_start(b5_sb[:, ob:ob + 1], b5_d[ob * 128:(ob + 1) * 128, :])
            cnt5 = float(B * N)
            for ob in range(8):
                nc.scalar.mul(mean5[:, ob:ob + 1], ar5_out[:, 2 * ob:2 * ob + 1], 1.0 / cnt5)
                nc.scalar.mul(e25[:, ob:ob + 1], ar5_out[:, 2 * ob + 1:2 * ob + 2], 1.0 / cnt5)
                nc.vector.tensor_tensor(out=var5[:, ob:ob + 1], in0=mean5[:, ob:ob + 1],
                                        in1=mean5[:, ob:ob + 1], op=Alu.mult)
                nc.vector.scalar_tensor_tensor(out=var5[:, ob:ob + 1], in0=var5[:, ob:ob + 1],
                                               scalar=-1.0, in1=e25[:, ob:ob + 1],
                                               op0=Alu.mult, op1=Alu.add)
                nc.vector.tensor_scalar_add(var5[:, ob:ob + 1], var5[:, ob:ob + 1], EPS)
                nc.vector.reciprocal(rec5[:, ob:ob + 1], var5[:, ob:ob + 1])
                nc.scalar.sqrt(rsq5[:, ob:ob + 1], rec5[:, ob:ob + 1])
                nc.vector.tensor_tensor(out=a5[:, ob:ob + 1], in0=g5_sb[:, ob:ob + 1],
                                        in1=rsq5[:, ob:ob + 1], op=Alu.mult)
                nc.vector.tensor_tensor(out=c5[:, ob:ob + 1], in0=mean5[:, ob:ob + 1],
                                        in1=a5[:, ob:ob + 1], op=Alu.mult)
                nc.vector.scalar_tensor_tensor(out=c5[:, ob:ob + 1], in0=c5[:, ob:ob + 1],
                                               scalar=-1.0, in1=b5_sb[:, ob:ob + 1],
                                               op0=Alu.mult, op1=Alu.add)
                z5 = pf.tile([128, N], dt.float32, tag="z5")
                nc.scalar.activation(z5, y_sb[ob], Act.Identity,
                                     bias=c5[:, ob:ob + 1], scale=a5[:, ob:ob + 1])
                o5 = pf.tile([128, N], dt.float32, tag="o5")
                nc.vector.scalar_tensor_tensor(out=o5, in0=z5, scalar=0.2, in1=z5,
                                               op0=Alu.mult, op1=Alu.max)
                nc.sync.dma_start(out_d[ob * 128:(ob + 1) * 128, :], o5)
            for i in range(4):
                nc.sync.dma_start(xdbg[i], h[i])

    nc.compile()
    return nc


def _get_compiled():
    if "nc" not in _CACHE:
        _CACHE["nc"] = _build()
    return _CACHE["nc"]


def _make_in_maps(inputs):
    x = np.ascontiguousarray(np.asarray(inputs["x"], dtype=np.float32))
    shared = {}
    Cs = [3, 64, 64, 128]
    for li in range(4):
        w = np.asarray(inputs[f"w{li+1}"], dtype=np.float32)
        C = Cs[li]
        shared[f"waT{li}"] = np.ascontiguousarray(w[:, :C].T)
        shared[f"wbmaT{li}"] = np.ascontiguousarray((w[:, C:] - w[:, :C]).T)
        shared[f"g{li}"] = np.ascontiguousarray(
            np.asarray(inputs[f"g{li+1}"], np.float32).reshape(-1, 1))
        shared[f"b{li}"] = np.ascontiguousarray(
            np.asarray(inputs[f"b{li+1}"], np.float32).reshape(-1, 1))
    shared["w5T"] = np.ascontiguousarray(np.asarray(inputs["w5"], np.float32).T)
    shared["g5"] = np.ascontiguousarray(np.asarray(inputs["g5"], np.float32).reshape(-1, 1))
    shared["b5"] = np.ascontiguousarray(np.asarray(inputs["b5"], np.float32).reshape(-1, 1))

    return [dict(shared, x0=np.ascontiguousarray(x[i])) for i in range(B)]


def kernel(**inputs):
    from concourse.bass_utils import run_bass_kernel_spmd

    nc = _get_compiled()
    in_maps = _make_in_maps(inputs)
    res = run_bass_kernel_spmd(nc, in_maps, core_ids=list(range(8)))
    out = np.stack([res.results[i]["out"] for i in range(B)]).astype(np.float32)
    return out


# revision 74
# speedup vs baseline: 1.0524x; 1.0524x over previous
"""DGCNN forward kernel for Trainium2 (8 NeuronCores, batch-parallel).

Strategy (per core = one sample of the batch), phase-major per layer to keep
engines streaming instead of the lockstep scores->lists->gather round-trips:
  - pre:    bf16 copies of x / weights, xx row, u = wa@x, v = (wb-wa)@x
            (bf16 matmuls, fp32 PSUM), u mean-centered.
  - phase1: for each of 16 row tiles: score matmul S = [x;1]^T[2x;-xx] (bf16,
            single PE op via augmentation when C<128), top-24 per row via
            3 rounds of DVE max8/find_index8/match_replace8, index lists
            transposed on PE into gather layout (group 0 only).
  - repl:   one batch of 14 SBUF DMAs replicates the index lists to the other
            16-partition groups (ap_gather wants per-core copies).
  - phase2: 64 ap_gathers (GPSIMD, runs ahead through a deep pool) + DVE
            segmented max/sum reduces + ACT square-accumulate for BN moments.
  - stats:  per-core BN moment terms, tiny AllReduce (syncBN), affine+lrelu.
  - final:  1x1 conv (bf16 PE) + BN + lrelu, stats via AllReduce.
"""

import numpy as np

B, C0, N = 8, 3, 2048
K = 20
EPS = 1e-5
LAYERS = [(3, 64), (64, 64), (64, 128), (128, 256)]  # (C_in, O)
NT = N // 128          # 16 row tiles
NCH = N // 512         # 4 matmul free-dim chunks
NCK = N // 64          # 32 gather chunks (64 points each)
CKR = 64               # points per gather chunk
NEG = -1.0e38

_CACHE = {}


def _build():
    import concourse.bass as bass
    import concourse.mybir as mybir
    from concourse import bacc
    from concourse.tile import TileContext

    dt = mybir.dt
    Alu = mybir.AluOpType
    Act = mybir.ActivationFunctionType

    nc = bacc.Bacc("TRN2", target_bir_lowering=False, debug=False,
                   enable_asserts=False, num_devices=8)

    # ---------------- DRAM I/O ----------------
    x_in = nc.dram_tensor("x0", [C0, N], dt.float32, kind="ExternalInput").ap()
    waT, wbmaT, gv, bv = {}, {}, {}, {}
    for li, (C, O) in enumerate(LAYERS):
        waT[li] = nc.dram_tensor(f"waT{li}", [C, O], dt.float32, kind="ExternalInput").ap()
        wbmaT[li] = nc.dram_tensor(f"wbmaT{li}", [C, O], dt.float32, kind="ExternalInput").ap()
        gv[li] = nc.dram_tensor(f"g{li}", [O, 1], dt.float32, kind="ExternalInput").ap()
        bv[li] = nc.dram_tensor(f"b{li}", [O, 1], dt.float32, kind="ExternalInput").ap()
    xdbg = [nc.dram_tensor(f"xdbg{i}", [128, N], dt.float32,
                           kind="ExternalOutput").ap() for i in range(4)]
    w5T_d = nc.dram_tensor("w5T", [512, 1024], dt.float32, kind="ExternalInput").ap()
    g5_d = nc.dram_tensor("g5", [1024, 1], dt.float32, kind="ExternalInput").ap()
    b5_d = nc.dram_tensor("b5", [1024, 1], dt.float32, kind="ExternalInput").ap()
    out_d = nc.dram_tensor("out", [1024, N], dt.float32, kind="ExternalOutput").ap()

    def sb(name, shape, dtype=dt.float32):
        return nc.alloc_sbuf_tensor(name, list(shape), dtype).ap()

    with TileContext(nc) as tc:
        # ---------------- persistent SBUF ----------------
        h = [sb("h0", [128, N]), sb("h1", [128, N]),
             sb("h2", [128, N]), sb("h3", [128, N])]
        ones_row = sb("ones_row", [1, N])
        nc.vector.memset(ones_row, 1.0)
        # identity for PE transpose: ident[p, f] = (f - p == 0)
        ident = sb("ident", [128, 128])
        iota_fp = sb("iota_fp", [128, 128], dt.int32)
        nc.gpsimd.iota(iota_fp, pattern=[[1, 128]], base=0, channel_multiplier=-1)
        nc.vector.tensor_scalar(out=ident, in0=iota_fp, scalar1=0, scalar2=None,
                                op0=Alu.is_equal)
        # replication selectors: REP1[c, o] = (c == o%16)  (16 nn slots),
        # REP2[c, o] = (c == 16 + (o%16)%4)  (slots 16-19 duplicated 4x)
        rep1 = sb("rep1", [24, 128])
        rep2 = sb("rep2", [24, 128])
        iota_r = sb("iota_r", [32, 128], dt.int32)
        nc.gpsimd.iota(iota_r, pattern=[[0, 8], [1, 16]], base=0,
                       channel_multiplier=-1)
        nc.vector.tensor_scalar(out=rep1, in0=iota_r[0:24, :], scalar1=0,
                                scalar2=None, op0=Alu.is_equal)
        nc.gpsimd.iota(iota_r, pattern=[[0, 32], [1, 4]], base=16,
                       channel_multiplier=-1)
        nc.vector.tensor_scalar(out=rep2, in0=iota_r[0:24, :], scalar1=0,
                                scalar2=None, op0=Alu.is_equal)

        # x2 needs its own base-partition-0 tensor (matmul operands share base)
        x2_sb = sb("x2_sb", [64, N])

        for li, (C, O) in enumerate(LAYERS):
            OT = (O + 127) // 128            # o-tiles
            ow = [min(128, O - ot * 128) for ot in range(OT)]
            AUG = C < 128
            CA = C + 1 if AUG else C
            CH = 128 if O >= 128 else 64     # gather channel count
            NG = CH // 16                    # 16-partition groups
            # 64-channel layers: replicate u into partitions 64-127 and give
            # the upper 4 Q7 cores the other half of the point stream, halving
            # the per-core gather index work (the layer bottleneck).
            SPLIT = CH == 64

            with tc.tile_pool(name=f"pp{li}", bufs=1) as pp, \
                 tc.tile_pool(name=f"ps{li}", bufs=4, space="PSUM") as psc, \
                 tc.tile_pool(name=f"pt{li}", bufs=4, space="PSUM") as psT2, \
                 tc.tile_pool(name=f"pg{li}", bufs=2) as pg, \
                 tc.tile_pool(name=f"pgs{li}", bufs=4) as pgs:

                def scope_in(s):
                    sid, _ = nc.enter_named_scope(f"L{li}_{s}", notify=False)
                    return (f"L{li}_{s}", sid)

                def scope_out(h_):
                    nc.leave_named_scope(h_[0], h_[1], notify=False)

                _sc = scope_in("pre")
                # ---- layer input (fp32, partitions 0..C-1) ----
                if li == 0:
                    xc = pp.tile([C0, N], dt.float32, tag="x0_sb")
                    nc.sync.dma_start(xc, x_in)
                elif li == 1:
                    xc = h[0][0:64, :]
                elif li == 2:
                    xc = x2_sb[:, :]
                else:
                    xc = h[1][:, :]

                # ---- score operands (fp32: bf16 is too coarse for the kNN
                # threshold region once |S| ~ C) ----
                ones_col = pp.tile([C, 1], dt.float32, tag="ones_col")
                nc.vector.memset(ones_col, 1.0)
                b2x = pp.tile([CA, N], dt.float32, tag="b2x")
                nc.scalar.mul(b2x[0:C, :], xc, 2.0)
                # xsq scratch lives in `up` (not yet written at this point)
                up = pp.tile([128, OT * N], dt.float32, tag="up")
                xsq = up[0:C, 0:N]
                nc.vector.tensor_tensor(out=xsq, in0=xc, in1=xc, op=Alu.mult)
                bnxx = pp.tile([1, N], dt.float32, tag="bnxx")
                for nch in range(NCH):
                    xxp = psc.tile([128, 512], dt.float32, tag="pt512")
                    nc.tensor.matmul(xxp[0:1, :], lhsT=ones_col,
                                     rhs=xsq[:, nch * 512:(nch + 1) * 512],
                                     start=True, stop=True)
                    nc.scalar.mul(bnxx[:, nch * 512:(nch + 1) * 512], xxp[0:1, :], -1.0)
                if AUG:
                    xaug = pp.tile([CA, N], dt.float32, tag="xaug")
                    nc.vector.tensor_copy(xaug[0:C, :], xc)
                    # rows at arbitrary partition base: fill via DMA
                    nc.sync.dma_start(xaug[C:C + 1, :], ones_row)
                    nc.sync.dma_start(b2x[C:C + 1, :], bnxx)

                # ---- u, v matmuls (fp32), centering ----
                waT_sb = pp.tile([C, O], dt.float32, tag="waT_sb")
                nc.sync.dma_start(waT_sb, waT[li])
                wbmaT_sb = pp.tile([C, O], dt.float32, tag="wbmaT_sb")
                nc.sync.dma_start(wbmaT_sb, wbmaT[li])

                vv = pp.tile([128, OT * N], dt.float32, tag="vv")
                sm = pp.tile([128, 96], dt.float32, tag="sm")
                neg_mu = sm[:, 0:2]
                sum_v = sm[:, 2:4]
                sum_u = sm[:, 4:6]
                sum_q = sm[:, 6:8]
                sum_s = sm[:, 8:10]
                svs = sm[:, 10:12]
                sv2 = sm[:, 12:14]
                vbar = sm[:, 14:16]
                beta = sm[:, 16:18]
                t1a = sm[:, 18:20]
                S1p = sm[:, 20:22]
                tA = sm[:, 22:24]
                tBt = sm[:, 24:26]
                S2p = sm[:, 26:28]
                tC = sm[:, 28:30]
                tD = sm[:, 30:32]
                mean = sm[:, 32:34]
                e2 = sm[:, 34:36]
                varp = sm[:, 36:38]
                rec = sm[:, 38:40]
                rsq = sm[:, 40:42]
                aco = sm[:, 42:44]
                bi = sm[:, 44:46]
                gsb = sm[:, 46:48]
                bsb = sm[:, 48:50]
                ar_in = sm[:, 50:54]
                ar_out = sm[:, 54:58]
                upart = sm[:, 58:58 + 2 * NCH]     # per-chunk accum partials
                vpart = sm[:, 58 + 2 * NCH:58 + 4 * NCH]

                for ot in range(OT):
                    w_ = ow[ot]
                    for nch in range(NCH):
                        upp = psc.tile([128, 512], dt.float32, tag="pt512")
                        nc.tensor.matmul(upp[0:w_, :],
                                         lhsT=waT_sb[:, ot * 128:ot * 128 + w_],
                                         rhs=xc[:, nch * 512:(nch + 1) * 512],
                                         start=True, stop=True)
                        nc.scalar.activation(
                            up[0:w_, ot * N + nch * 512: ot * N + (nch + 1) * 512],
                            upp[0:w_, :], Act.Copy)
                        vpp = psc.tile([128, 512], dt.float32, tag="pt512")
                        nc.tensor.matmul(vpp[0:w_, :],
                                         lhsT=wbmaT_sb[:, ot * 128:ot * 128 + w_],
                                         rhs=xc[:, nch * 512:(nch + 1) * 512],
                                         start=True, stop=True)
                        nc.scalar.activation(
                            vv[0:w_, ot * N + nch * 512: ot * N + (nch + 1) * 512],
                            vpp[0:w_, :], Act.Copy)
                    nc.vector.tensor_reduce(out=sum_u[0:w_, ot:ot + 1],
                                            in_=up[0:w_, ot * N:(ot + 1) * N],
                                            op=Alu.add, axis=mybir.AxisListType.X)
                    nc.vector.tensor_reduce(out=sum_v[0:w_, ot:ot + 1],
                                            in_=vv[0:w_, ot * N:(ot + 1) * N],
                                            op=Alu.add, axis=mybir.AxisListType.X)
                    nc.scalar.mul(neg_mu[0:w_, ot:ot + 1], sum_u[0:w_, ot:ot + 1],
                                  -1.0 / N)
                    nc.scalar.activation(up[0:w_, ot * N:(ot + 1) * N],
                                         up[0:w_, ot * N:(ot + 1) * N], Act.Identity,
                                         bias=neg_mu[0:w_, ot:ot + 1], scale=1.0)

                if SPLIT:
                    # replicate u so the upper 4 cores can gather the other
                    # half of the point stream in parallel (emitted before the
                    # list-replication DMAs so the sync queue can issue it
                    # before the first gathers need it)
                    nc.sync.dma_start(up[64:128, 0:N], up[0:64, 0:N])
                scope_out(_sc)
                _sc = scope_in("topk")
                # ---- phase 1: scores + topk + index lists ----
                # lg1[p, n] = If[n, p%16] (16 nn slots) is built replicated
                # across the 16-partition groups by one selector matmul.
                # lg2 holds slots 16-19 in the baseline 8-slot parity layout
                # (4 junk dups): even points in partitions 0-7, odd in 8-15
                # (staged via DMA), replicated per layer.
                # SPLIT layers: partitions 0-63 list the row tile's first 64
                # points, partitions 64-127 the other 64 (staged + DMA'd).
                PR = 64 if SPLIT else 128        # points listed per rt per half
                lg1 = pp.tile([128 if SPLIT else CH, NT * PR], dt.int16, tag="lg1")
                lg2 = pp.tile([128 if SPLIT else CH, NT * PR // 2], dt.int16, tag="lg2")
                st8odd = pp.tile([8, NT * PR // 2], dt.int16, tag="st8odd")
                if SPLIT:
                    stB1 = pp.tile([64, NT * 64], dt.int16, tag="stB1")
                    stBe = pp.tile([8, NT * 32], dt.int16, tag="stBe")
                    stBo = pp.tile([8, NT * 32], dt.int16, tag="stBo")
                tps = {}
                with tc.tile_pool(name=f"paS{li}",
                                  bufs=(3 if li in (1, 2) else 2)) as paS, \
                     tc.tile_pool(name=f"paI{li}", bufs=3) as paI:
                    def emit_scores_topk(rt):
                        Ssb = paS.tile([128, N], dt.float32, tag="Ssb")
                        for nch in range(NCH):
                            Spc = psc.tile([128, 512], dt.float32, tag="pt512")
                            if AUG:
                                nc.tensor.matmul(
                                    Spc, lhsT=xaug[:, rt * 128:(rt + 1) * 128],
                                    rhs=b2x[:, nch * 512:(nch + 1) * 512],
                                    start=True, stop=True)
                            else:
                                nc.tensor.matmul(
                                    Spc, lhsT=xc[:, rt * 128:(rt + 1) * 128],
                                    rhs=b2x[0:C, nch * 512:(nch + 1) * 512],
                                    start=True, stop=False)
                                nc.tensor.matmul(
                                    Spc, lhsT=ones_row[:, rt * 128:(rt + 1) * 128],
                                    rhs=bnxx[:, nch * 512:(nch + 1) * 512],
                                    start=False, stop=True)
                            nc.scalar.activation(Ssb[:, nch * 512:(nch + 1) * 512],
                                                 Spc, Act.Copy)
                        V = paI.tile([128, 24], dt.float32, tag="V")
                        I = paI.tile([128, 24], dt.uint16, tag="I")
                        Sw = paS.tile([128, N], dt.float32, tag="Ssb")
                        nc.vector.max(out=V[:, 0:8], in_=Ssb)
                        nc.vector.max_index(out=I[:, 0:8], in_max=V[:, 0:8],
                                            in_values=Ssb)
                        nc.vector.match_replace(out=Sw, in_to_replace=V[:, 0:8],
                                                in_values=Ssb, imm_value=NEG)
                        nc.vector.max(out=V[:, 8:16], in_=Sw)
                        nc.vector.max_index(out=I[:, 8:16], in_max=V[:, 8:16],
                                            in_values=Sw)
                        nc.vector.match_replace(out=Sw, in_to_replace=V[:, 8:16],
                                                in_values=Sw, imm_value=NEG)
                        nc.vector.max(out=V[:, 16:24], in_=Sw)
                        nc.vector.max_index(out=I[:, 16:24], in_max=V[:, 16:24],
                                            in_values=Sw)
                        If = paI.tile([128, 24], dt.float32, tag="If")
                        nc.vector.tensor_copy(If, I)
                        # transpose once, then replicate across 16-partition
                        # groups with constant selector matmuls (no DMAs)
                        t24 = psT2.tile([128, 128], dt.float32, tag="ptT")
                        nc.tensor.transpose(t24[0:24, :], If, ident)
                        IfT = paI.tile([24, 128], dt.float32, tag="IfT")
                        nc.vector.tensor_copy(IfT, t24[0:24, :])
                        if SPLIT:
                            tpa = psT2.tile([128, 128], dt.float32, tag="ptT")
                            nc.tensor.matmul(tpa[0:64, 0:64], lhsT=rep1[:, 0:64],
                                             rhs=IfT[:, 0:64], start=True, stop=True)
                            nc.tensor.matmul(tpa[0:64, 64:128], lhsT=rep1[:, 0:64],
                                             rhs=IfT[:, 64:128], start=True, stop=True)
                        else:
                            tpa = psT2.tile([128, 128], dt.float32, tag="ptT")
                            nc.tensor.matmul(tpa[0:CH, :], lhsT=rep1[:, 0:CH],
                                             rhs=IfT, start=True, stop=True)
                        tpb = psT2.tile([128, 128], dt.float32, tag="ptT")
                        nc.tensor.transpose(tpb[0:8, :], If[:, 16:24], ident)
                        return tpa, tpb

                    def emit_lists(rt, tpa, tpb):
                        if SPLIT:
                            nc.vector.tensor_copy(lg1[0:64, rt * 64:(rt + 1) * 64],
                                                  tpa[0:64, 0:64])
                            nc.vector.tensor_copy(stB1[:, rt * 64:(rt + 1) * 64],
                                                  tpa[0:64, 64:128])
                            nc.vector.tensor_copy(lg2[0:8, rt * 32:(rt + 1) * 32],
                                                  tpb[0:8, 0:64:2])
                            nc.vector.tensor_copy(st8odd[:, rt * 32:(rt + 1) * 32],
                                                  tpb[0:8, 1:64:2])
                            nc.vector.tensor_copy(stBe[:, rt * 32:(rt + 1) * 32],
                                                  tpb[0:8, 64:128:2])
                            nc.vector.tensor_copy(stBo[:, rt * 32:(rt + 1) * 32],
                                                  tpb[0:8, 65:128:2])
                            return
                        nc.vector.tensor_copy(lg1[:, rt * 128:(rt + 1) * 128],
                                              tpa[0:CH, :])
                        # baseline 8-slot parity layout for slots 16-19
                        nc.vector.tensor_copy(lg2[0:8, rt * 64:(rt + 1) * 64],
                                              tpb[0:8, 0::2])
                        nc.vector.tensor_copy(st8odd[:, rt * 64:(rt + 1) * 64],
                                              tpb[0:8, 1::2])

                    def repl_lists(hh):
                        # replicate/stage the lists for row tiles of half hh:
                        # emitting the first batch mid-topk lets the gathers
                        # start while the DVE is still on top-k of tiles 8-15
                        c1 = slice(hh * 8 * PR, (hh + 1) * 8 * PR)
                        c2 = slice(hh * 8 * (PR // 2), (hh + 1) * 8 * (PR // 2))
                        nc.sync.dma_start(lg2[8:16, c2], st8odd[:, c2])
                        if SPLIT:
                            nc.sync.dma_start(lg1[64:128, c1], stB1[:, c1])
                            nc.sync.dma_start(lg2[64:72, c2], stBe[:, c2])
                            nc.sync.dma_start(lg2[72:80, c2], stBo[:, c2])
                            for g in (1, 2, 3):
                                nc.sync.dma_start(lg2[g * 16:(g + 1) * 16, c2],
                                                  lg2[0:16, c2])
                                nc.sync.dma_start(
                                    lg2[64 + g * 16:64 + (g + 1) * 16, c2],
                                    lg2[64:80, c2])
                        else:
                            for g in range(1, NG):
                                nc.sync.dma_start(lg2[g * 16:(g + 1) * 16, c2],
                                                  lg2[0:16, c2])

                    for it in range(NT + 1):
                        if it - 1 >= 0:
                            emit_lists(it - 1, *tps.pop(it - 1))
                        if it == NT // 2:
                            repl_lists(0)
                        if it < NT:
                            tps[it] = emit_scores_topk(it)
                    repl_lists(1)

                scope_out(_sc)
                _sc = scope_in("gath")
                # ---- phase 2: gathers + reduces + BN moment accum ----
                # one g1 (16 slots) + one g2 (8-slot parity, 4 valid) gather
                # per row tile: gather cost scales with index count, so big
                # batches amortize the Q7 per-request overhead.
                mm = pp.tile([128, OT * N], dt.float32, tag="mm")
                s_sb = pp.tile([128, OT * N], dt.float32, tag="s_sb")
                qac = pp.tile([128, OT * 32], dt.float32, tag="qac")
                sqj = pp.tile([128, 2048], dt.float32, tag="sqj")
                if SPLIT:
                    msplit = pp.tile([128, NT * 64], dt.float32, tag="msplit")
                    ssplit = pp.tile([128, NT * 64], dt.float32, tag="ssplit")
                # pools opened at layer top: a pool boundary here would fence
                # the gathers behind all of phase 1
                if True:
                  if SPLIT:
                    w_ = 64
                    for r2 in range(NT // 2):
                        gA = pg.tile([128, 2048], dt.float32, tag="g1")
                        nc.gpsimd.ap_gather(
                            gA, up[:, 0:N], lg1[:, r2 * 128:(r2 + 1) * 128],
                            channels=128, num_elems=N, d=1, num_idxs=2048)
                        gB = pg.tile([128, 1024], dt.float32, tag="g2")
                        nc.gpsimd.ap_gather(
                            gB, up[:, 0:N], lg2[:, r2 * 64:(r2 + 1) * 64],
                            channels=128, num_elems=N, d=1, num_idxs=1024)
                        msl = slice(r2 * 128, (r2 + 1) * 128)
                        nc.vector.tensor_reduce(
                            out=msplit[:, msl],
                            in_=gA.rearrange("p (n k) -> p n k", k=16),
                            op=Alu.max, axis=mybir.AxisListType.X)
                        m2 = pgs.tile([128, 128], dt.float32, tag="m2")
                        nc.vector.tensor_reduce(
                            out=m2,
                            in_=gB.rearrange("p (n k) -> p n k", k=8)[:, :, 0:4],
                            op=Alu.max, axis=mybir.AxisListType.X)
                        nc.vector.tensor_tensor(out=msplit[:, msl],
                                                in0=msplit[:, msl],
                                                in1=m2, op=Alu.max)
                        s1 = pgs.tile([128, 128], dt.float32, tag="s1")
                        nc.vector.tensor_reduce(
                            out=s1,
                            in_=gA.rearrange("p (n k) -> p n k", k=16),
                            op=Alu.add, axis=mybir.AxisListType.X)
                        s2 = pgs.tile([128, 128], dt.float32, tag="s2")
                        nc.vector.tensor_reduce(
                            out=s2,
                            in_=gB.rearrange("p (n k) -> p n k", k=8)[:, :, 0:4],
                            op=Alu.add, axis=mybir.AxisListType.X)
                        nc.vector.tensor_tensor(out=ssplit[:, msl],
                                                in0=s1, in1=s2, op=Alu.add)
                        nc.scalar.activation(
                            sqj, gA, Act.Square,
                            accum_out=qac[:, 2 * r2:2 * r2 + 1])
                        g2v = gB.rearrange("p (n k) -> p n k", k=8)[:, :, 0:4]
                        sqv = sqj[:, 0:512].rearrange("p (n k) -> p n k", k=4)
                        nc.scalar.activation(
                            sqv, g2v, Act.Square,
                            accum_out=qac[:, 2 * r2 + 1:2 * r2 + 2])
                  else:
                    for ot in range(OT):
                        w_ = ow[ot]
                        wch = ((w_ + 15) // 16) * 16
                        usrc = up[0:wch, ot * N:(ot + 1) * N]
                        for rt in range(NT):
                            msl = slice(ot * N + rt * 128, ot * N + (rt + 1) * 128)
                            g1 = pg.tile([CH, 2048], dt.float32, tag="g1")
                            nc.gpsimd.ap_gather(
                                g1[0:wch, :], usrc,
                                lg1[0:wch, rt * 128: (rt + 1) * 128],
                                channels=wch, num_elems=N, d=1, num_idxs=2048)
                            g2 = pg.tile([CH, 1024], dt.float32, tag="g2")
                            nc.gpsimd.ap_gather(
                                g2[0:wch, :], usrc,
                                lg2[0:wch, rt * 64: (rt + 1) * 64],
                                channels=wch, num_elems=N, d=1, num_idxs=1024)
                            nc.vector.tensor_reduce(
                                out=mm[0:w_, msl],
                                in_=g1[0:w_, :].rearrange("p (n k) -> p n k", k=16),
                                op=Alu.max, axis=mybir.AxisListType.X)
                            m2 = pgs.tile([128, 128], dt.float32, tag="m2")
                            nc.vector.tensor_reduce(
                                out=m2[0:w_, :],
                                in_=g2[0:w_, :].rearrange("p (n k) -> p n k", k=8)[:, :, 0:4],
                                op=Alu.max, axis=mybir.AxisListType.X)
                            nc.vector.tensor_tensor(
                                out=mm[0:w_, msl], in0=mm[0:w_, msl],
                                in1=m2[0:w_, :], op=Alu.max)
                            s1 = pgs.tile([128, 128], dt.float32, tag="s1")
                            nc.vector.tensor_reduce(
                                out=s1[0:w_, :],
                                in_=g1[0:w_, :].rearrange("p (n k) -> p n k", k=16),
                                op=Alu.add, axis=mybir.AxisListType.X)
                            s2 = pgs.tile([128, 128], dt.float32, tag="s2")
                            nc.vector.tensor_reduce(
                                out=s2[0:w_, :],
                                in_=g2[0:w_, :].rearrange("p (n k) -> p n k", k=8)[:, :, 0:4],
                                op=Alu.add, axis=mybir.AxisListType.X)
                            nc.vector.tensor_tensor(
                                out=s_sb[0:w_, msl],
                                in0=s1[0:w_, :], in1=s2[0:w_, :], op=Alu.add)
                            # q: sum of u'^2 over the same neighbors (square to
                            # a scratch tile: in-place would WAR-couple the
                            # gathers to the scalar engine)
                            nc.scalar.activation(
                                sqj[0:w_, :], g1[0:w_, :], Act.Square,
                                accum_out=qac[0:w_, ot * 32 + 2 * rt: ot * 32 + 2 * rt + 1])
                            g2v = g2[0:w_, :].rearrange("p (n k) -> p n k", k=8)[:, :, 0:4]
                            sqv = sqj[0:w_, 0:512].rearrange("p (n k) -> p n k", k=4)
                            nc.scalar.activation(
                                sqv, g2v, Act.Square,
                                accum_out=qac[0:w_, ot * 32 + 2 * rt + 1: ot * 32 + 2 * rt + 2])
                if SPLIT:
                    # fold the upper-half results (points 64-127 of each row
                    # tile, computed at partitions 64-127) back down
                    nc.sync.dma_start(
                        mm[0:64, :].rearrange("p (r n) -> p r n", n=128)[:, :, 0:64],
                        msplit[0:64, :].rearrange("p (r n) -> p r n", n=64))
                    nc.sync.dma_start(
                        mm[0:64, :].rearrange("p (r n) -> p r n", n=128)[:, :, 64:128],
                        msplit[64:128, :].rearrange("p (r n) -> p r n", n=64))
                    nc.sync.dma_start(
                        s_sb[0:64, :].rearrange("p (r n) -> p r n", n=128)[:, :, 0:64],
                        ssplit[0:64, :].rearrange("p (r n) -> p r n", n=64))
                    nc.sync.dma_start(
                        s_sb[0:64, :].rearrange("p (r n) -> p r n", n=128)[:, :, 64:128],
                        ssplit[64:128, :].rearrange("p (r n) -> p r n", n=64))
                    nc.sync.dma_start(qac[0:64, 16:32], qac[64:128, 0:16])
                for ot in range(OT):
                    w_ = ow[ot]
                    nc.vector.tensor_reduce(out=sum_q[0:w_, ot:ot + 1],
                                            in_=qac[0:w_, ot * 32:(ot + 1) * 32],
                                            op=Alu.add, axis=mybir.AxisListType.X)

                scope_out(_sc)
                _sc = scope_in("stat")
                # ---- per-core stat terms + AllReduce ----
                for ot in range(OT):
                    w_ = ow[ot]
                    ssl = s_sb[0:w_, ot * N:(ot + 1) * N]
                    vsl = vv[0:w_, ot * N:(ot + 1) * N]
                    nc.vector.tensor_reduce(out=sum_s[0:w_, ot:ot + 1], in_=ssl,
                                            op=Alu.add, axis=mybir.AxisListType.X)
                    # up is fully consumed by the gathers at this point; reuse
                    # its slice as the accumulate-op scratch destination.
                    junk = up[:, ot * N:(ot + 1) * N]
                    nc.vector.scalar_tensor_tensor(out=junk[0:w_, :], in0=ssl, scalar=1.0,
                                                   in1=vsl, op0=Alu.mult, op1=Alu.mult,
                                                   accum_out=svs[0:w_, ot:ot + 1])
                    nc.vector.scalar_tensor_tensor(out=junk[0:w_, :], in0=vsl, scalar=1.0,
                                                   in1=vsl, op0=Alu.mult, op1=Alu.mult,
                                                   accum_out=sv2[0:w_, ot:ot + 1])
                    nc.scalar.mul(vbar[0:w_, ot:ot + 1], sum_v[0:w_, ot:ot + 1], 1.0 / N)
                    nc.vector.scalar_tensor_tensor(out=beta[0:w_, ot:ot + 1],
                                                   in0=neg_mu[0:w_, ot:ot + 1], scalar=-1.0,
                                                   in1=vbar[0:w_, ot:ot + 1],
                                                   op0=Alu.mult, op1=Alu.add)
                    nc.vector.scalar_tensor_tensor(out=t1a[0:w_, ot:ot + 1],
                                                   in0=vbar[0:w_, ot:ot + 1], scalar=-float(N),
                                                   in1=sum_v[0:w_, ot:ot + 1],
                                                   op0=Alu.mult, op1=Alu.add)
                    nc.vector.scalar_tensor_tensor(out=S1p[0:w_, ot:ot + 1],
                                                   in0=t1a[0:w_, ot:ot + 1], scalar=float(K),
                                                   in1=sum_s[0:w_, ot:ot + 1],
                                                   op0=Alu.mult, op1=Alu.add)
                    nc.vector.tensor_tensor(out=tA[0:w_, ot:ot + 1], in0=vbar[0:w_, ot:ot + 1],
                                            in1=sum_s[0:w_, ot:ot + 1], op=Alu.mult)
                    nc.vector.scalar_tensor_tensor(out=tA[0:w_, ot:ot + 1],
                                                   in0=tA[0:w_, ot:ot + 1], scalar=-1.0,
                                                   in1=svs[0:w_, ot:ot + 1],
                                                   op0=Alu.mult, op1=Alu.add)
                    nc.vector.tensor_tensor(out=tBt[0:w_, ot:ot + 1], in0=vbar[0:w_, ot:ot + 1],
                                            in1=vbar[0:w_, ot:ot + 1], op=Alu.mult)
                    nc.vector.scalar_tensor_tensor(out=tBt[0:w_, ot:ot + 1],
                                                   in0=tBt[0:w_, ot:ot + 1], scalar=-float(N),
                                                   in1=sv2[0:w_, ot:ot + 1],
                                                   op0=Alu.mult, op1=Alu.add)
                    nc.vector.scalar_tensor_tensor(out=S2p[0:w_, ot:ot + 1],
                                                   in0=tA[0:w_, ot:ot + 1], scalar=2.0,
                                                   in1=sum_q[0:w_, ot:ot + 1],
                                                   op0=Alu.mult, op1=Alu.add)
                    nc.vector.scalar_tensor_tensor(out=S2p[0:w_, ot:ot + 1],
                                                   in0=tBt[0:w_, ot:ot + 1], scalar=float(K),
                                                   in1=S2p[0:w_, ot:ot + 1],
                                                   op0=Alu.mult, op1=Alu.add)
                    cntl = float(N * K)
                    nc.vector.scalar_tensor_tensor(out=ar_in[0:w_, 2 * ot:2 * ot + 1],
                                                   in0=beta[0:w_, ot:ot + 1], scalar=cntl,
                                                   in1=S1p[0:w_, ot:ot + 1],
                                                   op0=Alu.mult, op1=Alu.add)
                    nc.vector.tensor_tensor(out=tC[0:w_, ot:ot + 1], in0=beta[0:w_, ot:ot + 1],
                                            in1=S1p[0:w_, ot:ot + 1], op=Alu.mult)
                    nc.vector.scalar_tensor_tensor(out=tC[0:w_, ot:ot + 1],
                                                   in0=tC[0:w_, ot:ot + 1], scalar=2.0,
                                                   in1=S2p[0:w_, ot:ot + 1],
                                                   op0=Alu.mult, op1=Alu.add)
                    nc.vector.tensor_tensor(out=tD[0:w_, ot:ot + 1], in0=beta[0:w_, ot:ot + 1],
                                            in1=beta[0:w_, ot:ot + 1], op=Alu.mult)
                    nc.vector.scalar_tensor_tensor(out=ar_in[0:w_, 2 * ot + 1:2 * ot + 2],
                                                   in0=tD[0:w_, ot:ot + 1], scalar=cntl,
                                                   in1=tC[0:w_, ot:ot + 1],
                                                   op0=Alu.mult, op1=Alu.add)

                with tc.tile_pool(name=f"dr{li}", bufs=1, space="DRAM") as dram:
                    ari = dram.tile([128, 2 * OT], dt.float32)
                    aro = dram.tile([128, 2 * OT], dt.float32)
                    nc.sync.dma_start(ari[:], ar_in[:, 0:2 * OT])
                    nc.gpsimd.collective_compute(
                        "AllReduce", Alu.add, replica_groups=[list(range(8))],
                        ins=[ari.opt()], outs=[aro.opt()])
                    nc.sync.dma_start(ar_out[:, 0:2 * OT], aro[:])
                # overlap with the AllReduce: mm <- m' + v (no AR dependency)
                for ot in range(OT):
                    w_ = ow[ot]
                    nc.vector.tensor_tensor(out=mm[0:w_, ot * N:(ot + 1) * N],
                                            in0=mm[0:w_, ot * N:(ot + 1) * N],
                                            in1=vv[0:w_, ot * N:(ot + 1) * N], op=Alu.add)

                # post-AR: mean/var/scale/bias + activation
                nc.sync.dma_start(gsb[0:ow[0], 0:1], gv[li][0:ow[0], :])
                nc.sync.dma_start(bsb[0:ow[0], 0:1], bv[li][0:ow[0], :])
                if OT > 1:
                    nc.sync.dma_start(gsb[0:ow[1], 1:2], gv[li][128:128 + ow[1], :])
                    nc.sync.dma_start(bsb[0:ow[1], 1:2], bv[li][128:128 + ow[1], :])
                cntg = float(B * N * K)
                for ot in range(OT):
                    w_ = ow[ot]
                    nc.scalar.mul(mean[0:w_, ot:ot + 1], ar_out[0:w_, 2 * ot:2 * ot + 1], 1.0 / cntg)
                    nc.scalar.mul(e2[0:w_, ot:ot + 1], ar_out[0:w_, 2 * ot + 1:2 * ot + 2], 1.0 / cntg)
                    nc.vector.tensor_tensor(out=varp[0:w_, ot:ot + 1], in0=mean[0:w_, ot:ot + 1],
                                            in1=mean[0:w_, ot:ot + 1], op=Alu.mult)
                    nc.vector.scalar_tensor_tensor(out=varp[0:w_, ot:ot + 1],
                                                   in0=varp[0:w_, ot:ot + 1], scalar=-1.0,
                                                   in1=e2[0:w_, ot:ot + 1],
                                                   op0=Alu.mult, op1=Alu.add)
                    nc.vector.tensor_scalar_add(varp[0:w_, ot:ot + 1], varp[0:w_, ot:ot + 1], EPS)
                    nc.vector.reciprocal(rec[0:w_, ot:ot + 1], varp[0:w_, ot:ot + 1])
                    nc.scalar.sqrt(rsq[0:w_, ot:ot + 1], rec[0:w_, ot:ot + 1])
                    nc.vector.tensor_tensor(out=aco[0:w_, ot:ot + 1], in0=gsb[0:w_, ot:ot + 1],
                                            in1=rsq[0:w_, ot:ot + 1], op=Alu.mult)
                    nc.vector.tensor_tensor(out=bi[0:w_, ot:ot + 1], in0=neg_mu[0:w_, ot:ot + 1],
                                            in1=mean[0:w_, ot:ot + 1], op=Alu.add)
                    nc.vector.tensor_tensor(out=bi[0:w_, ot:ot + 1], in0=bi[0:w_, ot:ot + 1],
                                            in1=aco[0:w_, ot:ot + 1], op=Alu.mult)
                    nc.vector.scalar_tensor_tensor(out=bi[0:w_, ot:ot + 1],
                                                   in0=bi[0:w_, ot:ot + 1], scalar=-1.0,
                                                   in1=bsb[0:w_, ot:ot + 1],
                                                   op0=Alu.mult, op1=Alu.add)
                    if li == 0:
                        dst = h[0][0:64, :]
                    elif li == 1:
                        dst = x2_sb[:, :]
                    elif li == 2:
                        dst = h[1][:, :]
                    else:
                        dst = h[2][:, :] if ot == 0 else h[3][:, :]
                    zt = up[:, ot * N:(ot + 1) * N]
                    nc.scalar.activation(zt[0:w_, :], mm[0:w_, ot * N:(ot + 1) * N],
                                         Act.Identity,
                                         bias=bi[0:w_, ot:ot + 1], scale=aco[0:w_, ot:ot + 1])
                    nc.vector.scalar_tensor_tensor(out=dst[0:w_, :],
                                                   in0=zt[0:w_, :], scalar=0.2,
                                                   in1=zt[0:w_, :], op0=Alu.mult, op1=Alu.max)
                    if li == 1:
                        nc.sync.dma_start(h[0][64:128, :], x2_sb)
                scope_out(_sc)

        # ---------------- final conv + BN + lrelu ----------------
        with tc.tile_pool(name="pf", bufs=1) as pf, \
             tc.tile_pool(name="pfp", bufs=2, space="PSUM") as pfp:
            y_sb = []
            for ob in range(8):
                ytile = pf.tile([128, N], dt.float32, tag=f"y{ob}")
                y_sb.append(ytile)
            sm5 = pf.tile([128, 160], dt.float32, tag="sm5")
            sum_y = sm5[:, 0:8]
            mu5 = sm5[:, 8:16]
            nmu5 = sm5[:, 16:24]
            syc2 = sm5[:, 24:32]
            tE = sm5[:, 32:40]
            tF = sm5[:, 40:48]
            g5_sb = sm5[:, 48:56]
            b5_sb = sm5[:, 56:64]
            mean5 = sm5[:, 64:72]
            e25 = sm5[:, 72:80]
            var5 = sm5[:, 80:88]
            rec5 = sm5[:, 88:96]
            rsq5 = sm5[:, 96:104]
            a5 = sm5[:, 104:112]
            c5 = sm5[:, 112:120]
            ar5_in = sm5[:, 120:136]
            ar5_out = sm5[:, 136:152]
            w5T_sb = pf.tile([128, 4 * 1024], dt.float32, tag="w5T_sb")
            for cb in range(4):
                nc.sync.dma_start(w5T_sb[:, cb * 1024:(cb + 1) * 1024],
                                  w5T_d[cb * 128:(cb + 1) * 128, :])
            w5b = pf.tile([128, 4 * 1024], dt.bfloat16, tag="w5b")
            nc.vector.tensor_copy(w5b, w5T_sb)
            hb = []
            for cb in range(4):
                hbt = pf.tile([128, N], dt.bfloat16, tag=f"hb{cb}")
                nc.vector.tensor_copy(hbt, h[cb])
                hb.append(hbt)
            for ob in range(8):
                for nch in range(NCH):
                    yp = pfp.tile([128, 512], dt.float32, tag="yp")
                    for cb in range(4):
                        nc.tensor.matmul(yp, lhsT=w5b[:, cb * 1024 + ob * 128:
                                                      cb * 1024 + (ob + 1) * 128],
                                         rhs=hb[cb][:, nch * 512:(nch + 1) * 512],
                                         start=(cb == 0), stop=(cb == 3))
                    nc.scalar.activation(y_sb[ob][:, nch * 512:(nch + 1) * 512], yp, Act.Copy)
                nc.vector.tensor_reduce(out=sum_y[:, ob:ob + 1], in_=y_sb[ob],
                                        op=Alu.add, axis=mybir.AxisListType.X)
                nc.scalar.mul(mu5[:, ob:ob + 1], sum_y[:, ob:ob + 1], 1.0 / N)
                nc.scalar.mul(nmu5[:, ob:ob + 1], sum_y[:, ob:ob + 1], -1.0 / N)
                yc = pf.tile([128, N], dt.float32, tag="yc")
                nc.scalar.activation(yc, y_sb[ob], Act.Identity,
                                     bias=nmu5[:, ob:ob + 1], scale=1.0)
                junk5 = pf.tile([128, N], dt.float32, tag="junk5")
                nc.vector.scalar_tensor_tensor(out=junk5, in0=yc, scalar=1.0, in1=yc,
                                               op0=Alu.mult, op1=Alu.mult,
                                               accum_out=syc2[:, ob:ob + 1])
                nc.vector.tensor_copy(ar5_in[:, 2 * ob:2 * ob + 1], sum_y[:, ob:ob + 1])
                nc.vector.tensor_tensor(out=tE[:, ob:ob + 1], in0=mu5[:, ob:ob + 1],
                                        in1=sum_y[:, ob:ob + 1], op=Alu.mult)
                nc.vector.scalar_tensor_tensor(out=tE[:, ob:ob + 1], in0=tE[:, ob:ob + 1],
                                               scalar=2.0, in1=syc2[:, ob:ob + 1],
                                               op0=Alu.mult, op1=Alu.add)
                nc.vector.tensor_tensor(out=tF[:, ob:ob + 1], in0=mu5[:, ob:ob + 1],
                                        in1=mu5[:, ob:ob + 1], op=Alu.mult)
                nc.vector.scalar_tensor_tensor(out=ar5_in[:, 2 * ob + 1:2 * ob + 2],
                                               in0=tF[:, ob:ob + 1], scalar=-float(N),
                                               in1=tE[:, ob:ob + 1],
                                               op0=Alu.mult, op1=Alu.add)
            with tc.tile_pool(name="dr5", bufs=1, space="DRAM") as dram5:
                ari5 = dram5.tile([128, 16], dt.float32)
                aro5 = dram5.tile([128, 16], dt.float32)
                nc.sync.dma_start(ari5[:], ar5_in)
                nc.gpsimd.collective_compute(
                    "AllReduce", Alu.add, replica_groups=[list(range(8))],
                    ins=[ari5.opt()], outs=[aro5.opt()])
                nc.sync.dma_start(ar5_out, aro5[:])
            for ob in range(8):
                nc.sync.dma_start(g5_sb[:, ob:ob + 1], g5_d[ob * 128:(ob + 1) * 128, :])
                nc.sync.dma_start(b5_sb[:, ob:ob + 1], b5_d[ob * 128:(ob + 1) * 128, :])
            cnt5 = float(B * N)
            for ob in range(8):
                nc.scalar.mul(mean5[:, ob:ob + 1], ar5_out[:, 2 * ob:2 * ob + 1], 1.0 / cnt5)
                nc.scalar.mul(e25[:, ob:ob + 1], ar5_out[:, 2 * ob + 1:2 * ob + 2], 1.0 / cnt5)
                nc.vector.tensor_tensor(out=var5[:, ob:ob + 1], in0=mean5[:, ob:ob + 1],
                                        in1=mean5[:, ob:ob + 1], op=Alu.mult)
                nc.vector.scalar_tensor_tensor(out=var5[:, ob:ob + 1], in0=var5[:, ob:ob + 1],
                                               scalar=-1.0, in1=e25[:, ob:ob + 1],
                                               op0=Alu.mult, op1=Alu.add)
                nc.vector.tensor_scalar_add(var5[:, ob:ob + 1], var5[:, ob:ob + 1], EPS)
                nc.vector.reciprocal(rec5[:, ob:ob + 1], var5[:, ob:ob + 1])
                nc.scalar.sqrt(rsq5[:, ob:ob + 1], rec5[:, ob:ob + 1])
                nc.vector.tensor_tensor(out=a5[:, ob:ob + 1], in0=g5_sb[:, ob:ob + 1],
                                        in1=rsq5[:, ob:ob + 1], op=Alu.mult)
                nc.vector.tensor_tensor(out=c5[:, ob:ob + 1], in0=mean5[:, ob:ob + 1],
                                        in1=a5[:, ob:ob + 1], op=Alu.mult)
                nc.vector.scalar_tensor_tensor(out=c5[:, ob:ob + 1], in0=c5[:, ob:ob + 1],
                                               scalar=-1.0, in1=b5_sb[:, ob:ob + 1],
                                               op0=Alu.mult, op1=Alu.add)
                z5 = pf.tile([128, N], dt.float32, tag="z5")
                nc.scalar.activation(z5, y_sb[ob], Act.Identity,
                                     bias=c5[:, ob:ob + 1], scale=a5[:, ob:ob + 1])
                o5 = pf.tile([128, N], dt.float32, tag="o5")
                nc.vector.scalar_tensor_tensor(out=o5, in0=z5, scalar=0.2, in1=z5,
                                               op0=Alu.mult, op1=Alu.max)
                nc.sync.dma_start(out_d[ob * 128:(ob + 1) * 128, :], o5)
            for i in range(4):
                nc.sync.dma_start(xdbg[i], h[i])

    nc.compile()
    return nc


def _get_compiled():
    if "nc" not in _CACHE:
        _CACHE["nc"] = _build()
    return _CACHE["nc"]


def _make_in_maps(inputs):
    x = np.ascontiguousarray(np.asarray(inputs["x"], dtype=np.float32))
    shared = {}
    Cs = [3, 64, 64, 128]
    for li in range(4):
        w = np.asarray(inputs[f"w{li+1}"], dtype=np.float32)
        C = Cs[li]
        shared[f"waT{li}"] = np.ascontiguousarray(w[:, :C].T)
        shared[f"wbmaT{li}"] = np.ascontiguousarray((w[:, C:] - w[:, :C]).T)
        shared[f"g{li}"] = np.ascontiguousarray(
            np.asarray(inputs[f"g{li+1}"], np.float32).reshape(-1, 1))
        shared[f"b{li}"] = np.ascontiguousarray(
            np.asarray(inputs[f"b{li+1}"], np.float32).reshape(-1, 1))
    shared["w5T"] = np.ascontiguousarray(np.asarray(inputs["w5"], np.float32).T)
    shared["g5"] = np.ascontiguousarray(np.asarray(inputs["g5"], np.float32).reshape(-1, 1))
    shared["b5"] = np.ascontiguousarray(np.asarray(inputs["b5"], np.float32).reshape(-1, 1))

    return [dict(shared, x0=np.ascontiguousarray(x[i])) for i in range(B)]


def kernel(**inputs):
    from concourse.bass_utils import run_bass_kernel_spmd

    nc = _get_compiled()
    in_maps = _make_in_maps(inputs)
    res = run_bass_kernel_spmd(nc, in_maps, core_ids=list(range(8)))
    out = np.stack([res.results[i]["out"] for i in range(B)]).astype(np.float32)
    return out


# revision 75
# speedup vs baseline: 1.0691x; 1.0159x over previous
"""DGCNN forward kernel for Trainium2 (8 NeuronCores, batch-parallel).

Strategy (per core = one sample of the batch), phase-major per layer to keep
engines streaming instead of the lockstep scores->lists->gather round-trips:
  - pre:    bf16 copies of x / weights, xx row, u = wa@x, v = (wb-wa)@x
            (bf16 matmuls, fp32 PSUM), u mean-centered.
  - phase1: for each of 16 row tiles: score matmul S = [x;1]^T[2x;-xx] (bf16,
            single PE op via augmentation when C<128), top-24 per row via
            3 rounds of DVE max8/find_index8/match_replace8, index lists
            transposed on PE into gather layout (group 0 only).
  - repl:   one batch of 14 SBUF DMAs replicates the index lists to the other
            16-partition groups (ap_gather wants per-core copies).
  - phase2: 64 ap_gathers (GPSIMD, runs ahead through a deep pool) + DVE
            segmented max/sum reduces + ACT square-accumulate for BN moments.
  - stats:  per-core BN moment terms, tiny AllReduce (syncBN), affine+lrelu.
  - final:  1x1 conv (bf16 PE) + BN + lrelu, stats via AllReduce.
"""

import numpy as np

B, C0, N = 8, 3, 2048
K = 20
EPS = 1e-5
LAYERS = [(3, 64), (64, 64), (64, 128), (128, 256)]  # (C_in, O)
NT = N // 128          # 16 row tiles
NCH = N // 512         # 4 matmul free-dim chunks
NCK = N // 64          # 32 gather chunks (64 points each)
CKR = 64               # points per gather chunk
NEG = -1.0e38

_CACHE = {}


def _build():
    import concourse.bass as bass
    import concourse.mybir as mybir
    from concourse import bacc
    from concourse.tile import TileContext

    dt = mybir.dt
    Alu = mybir.AluOpType
    Act = mybir.ActivationFunctionType

    nc = bacc.Bacc("TRN2", target_bir_lowering=False, debug=False,
                   enable_asserts=False, num_devices=8)

    # ---------------- DRAM I/O ----------------
    x_in = nc.dram_tensor("x0", [C0, N], dt.float32, kind="ExternalInput").ap()
    waT, wbmaT, gv, bv = {}, {}, {}, {}
    for li, (C, O) in enumerate(LAYERS):
        waT[li] = nc.dram_tensor(f"waT{li}", [C, O], dt.float32, kind="ExternalInput").ap()
        wbmaT[li] = nc.dram_tensor(f"wbmaT{li}", [C, O], dt.float32, kind="ExternalInput").ap()
        gv[li] = nc.dram_tensor(f"g{li}", [O, 1], dt.float32, kind="ExternalInput").ap()
        bv[li] = nc.dram_tensor(f"b{li}", [O, 1], dt.float32, kind="ExternalInput").ap()
    xdbg = [nc.dram_tensor(f"xdbg{i}", [128, N], dt.float32,
                           kind="ExternalOutput").ap() for i in range(4)]
    w5T_d = nc.dram_tensor("w5T", [512, 1024], dt.float32, kind="ExternalInput").ap()
    g5_d = nc.dram_tensor("g5", [1024, 1], dt.float32, kind="ExternalInput").ap()
    b5_d = nc.dram_tensor("b5", [1024, 1], dt.float32, kind="ExternalInput").ap()
    out_d = nc.dram_tensor("out", [1024, N], dt.float32, kind="ExternalOutput").ap()

    def sb(name, shape, dtype=dt.float32):
        return nc.alloc_sbuf_tensor(name, list(shape), dtype).ap()

    with TileContext(nc) as tc:
        # ---------------- persistent SBUF ----------------
        h = [sb("h0", [128, N]), sb("h1", [128, N]),
             sb("h2", [128, N]), sb("h3", [128, N])]
        ones_row = sb("ones_row", [1, N])
        nc.vector.memset(ones_row, 1.0)
        # identity for PE transpose: ident[p, f] = (f - p == 0)
        ident = sb("ident", [128, 128])
        iota_fp = sb("iota_fp", [128, 128], dt.int32)
        nc.gpsimd.iota(iota_fp, pattern=[[1, 128]], base=0, channel_multiplier=-1)
        nc.vector.tensor_scalar(out=ident, in0=iota_fp, scalar1=0, scalar2=None,
                                op0=Alu.is_equal)
        # replication selectors: REP1[c, o] = (c == o%16)  (16 nn slots),
        # REP2[c, o] = (c == 16 + (o%16)%4)  (slots 16-19 duplicated 4x)
        rep1 = sb("rep1", [24, 128])
        rep2 = sb("rep2", [24, 128])
        iota_r = sb("iota_r", [32, 128], dt.int32)
        nc.gpsimd.iota(iota_r, pattern=[[0, 8], [1, 16]], base=0,
                       channel_multiplier=-1)
        nc.vector.tensor_scalar(out=rep1, in0=iota_r[0:24, :], scalar1=0,
                                scalar2=None, op0=Alu.is_equal)
        nc.gpsimd.iota(iota_r, pattern=[[0, 32], [1, 4]], base=16,
                       channel_multiplier=-1)
        nc.vector.tensor_scalar(out=rep2, in0=iota_r[0:24, :], scalar1=0,
                                scalar2=None, op0=Alu.is_equal)

        # x2 needs its own base-partition-0 tensor (matmul operands share base)
        x2_sb = sb("x2_sb", [64, N])

        for li, (C, O) in enumerate(LAYERS):
            OT = (O + 127) // 128            # o-tiles
            ow = [min(128, O - ot * 128) for ot in range(OT)]
            AUG = C < 128
            CA = C + 1 if AUG else C
            CH = 128 if O >= 128 else 64     # gather channel count
            NG = CH // 16                    # 16-partition groups
            # 64-channel layers: replicate u into partitions 64-127 and give
            # the upper 4 Q7 cores the other half of the point stream, halving
            # the per-core gather index work (the layer bottleneck).
            SPLIT = CH == 64

            with tc.tile_pool(name=f"pp{li}", bufs=1) as pp, \
                 tc.tile_pool(name=f"ps{li}", bufs=4, space="PSUM") as psc, \
                 tc.tile_pool(name=f"pt{li}", bufs=4, space="PSUM") as psT2, \
                 tc.tile_pool(name=f"pg{li}", bufs=(2 if li == 3 else 3)) as pg, \
                 tc.tile_pool(name=f"pgs{li}", bufs=4) as pgs:

                def scope_in(s):
                    sid, _ = nc.enter_named_scope(f"L{li}_{s}", notify=False)
                    return (f"L{li}_{s}", sid)

                def scope_out(h_):
                    nc.leave_named_scope(h_[0], h_[1], notify=False)

                _sc = scope_in("pre")
                # ---- layer input (fp32, partitions 0..C-1) ----
                if li == 0:
                    xc = pp.tile([C0, N], dt.float32, tag="x0_sb")
                    nc.sync.dma_start(xc, x_in)
                elif li == 1:
                    xc = h[0][0:64, :]
                elif li == 2:
                    xc = x2_sb[:, :]
                else:
                    xc = h[1][:, :]

                # ---- score operands (fp32: bf16 is too coarse for the kNN
                # threshold region once |S| ~ C) ----
                ones_col = pp.tile([C, 1], dt.float32, tag="ones_col")
                nc.vector.memset(ones_col, 1.0)
                b2x = pp.tile([CA, N], dt.float32, tag="b2x")
                nc.scalar.mul(b2x[0:C, :], xc, 2.0)
                # xsq scratch lives in `up` (not yet written at this point)
                up = pp.tile([128, OT * N], dt.float32, tag="up")
                xsq = up[0:C, 0:N]
                nc.vector.tensor_tensor(out=xsq, in0=xc, in1=xc, op=Alu.mult)
                bnxx = pp.tile([1, N], dt.float32, tag="bnxx")
                for nch in range(NCH):
                    xxp = psc.tile([128, 512], dt.float32, tag="pt512")
                    nc.tensor.matmul(xxp[0:1, :], lhsT=ones_col,
                                     rhs=xsq[:, nch * 512:(nch + 1) * 512],
                                     start=True, stop=True)
                    nc.scalar.mul(bnxx[:, nch * 512:(nch + 1) * 512], xxp[0:1, :], -1.0)
                if AUG:
                    xaug = pp.tile([CA, N], dt.float32, tag="xaug")
                    nc.vector.tensor_copy(xaug[0:C, :], xc)
                    # rows at arbitrary partition base: fill via DMA
                    nc.sync.dma_start(xaug[C:C + 1, :], ones_row)
                    nc.sync.dma_start(b2x[C:C + 1, :], bnxx)

                # ---- u, v matmuls (fp32), centering ----
                waT_sb = pp.tile([C, O], dt.float32, tag="waT_sb")
                nc.sync.dma_start(waT_sb, waT[li])
                wbmaT_sb = pp.tile([C, O], dt.float32, tag="wbmaT_sb")
                nc.sync.dma_start(wbmaT_sb, wbmaT[li])

                vv = pp.tile([128, OT * N], dt.float32, tag="vv")
                sm = pp.tile([128, 96], dt.float32, tag="sm")
                neg_mu = sm[:, 0:2]
                sum_v = sm[:, 2:4]
                sum_u = sm[:, 4:6]
                sum_q = sm[:, 6:8]
                sum_s = sm[:, 8:10]
                svs = sm[:, 10:12]
                sv2 = sm[:, 12:14]
                vbar = sm[:, 14:16]
                beta = sm[:, 16:18]
                t1a = sm[:, 18:20]
                S1p = sm[:, 20:22]
                tA = sm[:, 22:24]
                tBt = sm[:, 24:26]
                S2p = sm[:, 26:28]
                tC = sm[:, 28:30]
                tD = sm[:, 30:32]
                mean = sm[:, 32:34]
                e2 = sm[:, 34:36]
                varp = sm[:, 36:38]
                rec = sm[:, 38:40]
                rsq = sm[:, 40:42]
                aco = sm[:, 42:44]
                bi = sm[:, 44:46]
                gsb = sm[:, 46:48]
                bsb = sm[:, 48:50]
                ar_in = sm[:, 50:54]
                ar_out = sm[:, 54:58]
                upart = sm[:, 58:58 + 2 * NCH]     # per-chunk accum partials
                vpart = sm[:, 58 + 2 * NCH:58 + 4 * NCH]

                for ot in range(OT):
                    w_ = ow[ot]
                    for nch in range(NCH):
                        upp = psc.tile([128, 512], dt.float32, tag="pt512")
                        nc.tensor.matmul(upp[0:w_, :],
                                         lhsT=waT_sb[:, ot * 128:ot * 128 + w_],
                                         rhs=xc[:, nch * 512:(nch + 1) * 512],
                                         start=True, stop=True)
                        nc.scalar.activation(
                            up[0:w_, ot * N + nch * 512: ot * N + (nch + 1) * 512],
                            upp[0:w_, :], Act.Copy)
                        vpp = psc.tile([128, 512], dt.float32, tag="pt512")
                        nc.tensor.matmul(vpp[0:w_, :],
                                         lhsT=wbmaT_sb[:, ot * 128:ot * 128 + w_],
                                         rhs=xc[:, nch * 512:(nch + 1) * 512],
                                         start=True, stop=True)
                        nc.scalar.activation(
                            vv[0:w_, ot * N + nch * 512: ot * N + (nch + 1) * 512],
                            vpp[0:w_, :], Act.Copy)
                    nc.vector.tensor_reduce(out=sum_u[0:w_, ot:ot + 1],
                                            in_=up[0:w_, ot * N:(ot + 1) * N],
                                            op=Alu.add, axis=mybir.AxisListType.X)
                    nc.vector.tensor_reduce(out=sum_v[0:w_, ot:ot + 1],
                                            in_=vv[0:w_, ot * N:(ot + 1) * N],
                                            op=Alu.add, axis=mybir.AxisListType.X)
                    nc.scalar.mul(neg_mu[0:w_, ot:ot + 1], sum_u[0:w_, ot:ot + 1],
                                  -1.0 / N)
                    nc.scalar.activation(up[0:w_, ot * N:(ot + 1) * N],
                                         up[0:w_, ot * N:(ot + 1) * N], Act.Identity,
                                         bias=neg_mu[0:w_, ot:ot + 1], scale=1.0)

                if SPLIT:
                    # replicate u so the upper 4 cores can gather the other
                    # half of the point stream in parallel (emitted before the
                    # list-replication DMAs so the sync queue can issue it
                    # before the first gathers need it)
                    nc.sync.dma_start(up[64:128, 0:N], up[0:64, 0:N])
                scope_out(_sc)
                _sc = scope_in("topk")
                # ---- phase 1: scores + topk + index lists ----
                # lg1[p, n] = If[n, p%16] (16 nn slots) is built replicated
                # across the 16-partition groups by one selector matmul.
                # lg2 holds slots 16-19 in the baseline 8-slot parity layout
                # (4 junk dups): even points in partitions 0-7, odd in 8-15
                # (staged via DMA), replicated per layer.
                # SPLIT layers: partitions 0-63 list the row tile's first 64
                # points, partitions 64-127 the other 64 (staged + DMA'd).
                PR = 64 if SPLIT else 128        # points listed per rt per half
                lg1 = pp.tile([128 if SPLIT else CH, NT * PR], dt.int16, tag="lg1")
                lg2 = pp.tile([128 if SPLIT else CH, NT * PR // 2], dt.int16, tag="lg2")
                st8odd = pp.tile([8, NT * PR // 2], dt.int16, tag="st8odd")
                if SPLIT:
                    stB1 = pp.tile([64, NT * 64], dt.int16, tag="stB1")
                    stBe = pp.tile([8, NT * 32], dt.int16, tag="stBe")
                    stBo = pp.tile([8, NT * 32], dt.int16, tag="stBo")
                tps = {}
                with tc.tile_pool(name=f"paS{li}",
                                  bufs=(3 if li in (1, 2) else 2)) as paS, \
                     tc.tile_pool(name=f"paI{li}", bufs=3) as paI:
                    def emit_scores_topk(rt):
                        Ssb = paS.tile([128, N], dt.float32, tag="Ssb")
                        for nch in range(NCH):
                            Spc = psc.tile([128, 512], dt.float32, tag="pt512")
                            if AUG:
                                nc.tensor.matmul(
                                    Spc, lhsT=xaug[:, rt * 128:(rt + 1) * 128],
                                    rhs=b2x[:, nch * 512:(nch + 1) * 512],
                                    start=True, stop=True)
                            else:
                                nc.tensor.matmul(
                                    Spc, lhsT=xc[:, rt * 128:(rt + 1) * 128],
                                    rhs=b2x[0:C, nch * 512:(nch + 1) * 512],
                                    start=True, stop=False)
                                nc.tensor.matmul(
                                    Spc, lhsT=ones_row[:, rt * 128:(rt + 1) * 128],
                                    rhs=bnxx[:, nch * 512:(nch + 1) * 512],
                                    start=False, stop=True)
                            nc.scalar.activation(Ssb[:, nch * 512:(nch + 1) * 512],
                                                 Spc, Act.Copy)
                        V = paI.tile([128, 24], dt.float32, tag="V")
                        I = paI.tile([128, 24], dt.uint16, tag="I")
                        Sw = paS.tile([128, N], dt.float32, tag="Ssb")
                        nc.vector.max(out=V[:, 0:8], in_=Ssb)
                        nc.vector.max_index(out=I[:, 0:8], in_max=V[:, 0:8],
                                            in_values=Ssb)
                        nc.vector.match_replace(out=Sw, in_to_replace=V[:, 0:8],
                                                in_values=Ssb, imm_value=NEG)
                        nc.vector.max(out=V[:, 8:16], in_=Sw)
                        nc.vector.max_index(out=I[:, 8:16], in_max=V[:, 8:16],
                                            in_values=Sw)
                        nc.vector.match_replace(out=Sw, in_to_replace=V[:, 8:16],
                                                in_values=Sw, imm_value=NEG)
                        nc.vector.max(out=V[:, 16:24], in_=Sw)
                        nc.vector.max_index(out=I[:, 16:24], in_max=V[:, 16:24],
                                            in_values=Sw)
                        If = paI.tile([128, 24], dt.float32, tag="If")
                        nc.vector.tensor_copy(If, I)
                        # transpose once, then replicate across 16-partition
                        # groups with constant selector matmuls (no DMAs)
                        t24 = psT2.tile([128, 128], dt.float32, tag="ptT")
                        nc.tensor.transpose(t24[0:24, :], If, ident)
                        IfT = paI.tile([24, 128], dt.float32, tag="IfT")
                        nc.vector.tensor_copy(IfT, t24[0:24, :])
                        if SPLIT:
                            tpa = psT2.tile([128, 128], dt.float32, tag="ptT")
                            nc.tensor.matmul(tpa[0:64, 0:64], lhsT=rep1[:, 0:64],
                                             rhs=IfT[:, 0:64], start=True, stop=True)
                            nc.tensor.matmul(tpa[0:64, 64:128], lhsT=rep1[:, 0:64],
                                             rhs=IfT[:, 64:128], start=True, stop=True)
                        else:
                            tpa = psT2.tile([128, 128], dt.float32, tag="ptT")
                            nc.tensor.matmul(tpa[0:CH, :], lhsT=rep1[:, 0:CH],
                                             rhs=IfT, start=True, stop=True)
                        tpb = psT2.tile([128, 128], dt.float32, tag="ptT")
                        nc.tensor.transpose(tpb[0:8, :], If[:, 16:24], ident)
                        return tpa, tpb

                    def emit_lists(rt, tpa, tpb):
                        if SPLIT:
                            nc.vector.tensor_copy(lg1[0:64, rt * 64:(rt + 1) * 64],
                                                  tpa[0:64, 0:64])
                            nc.vector.tensor_copy(stB1[:, rt * 64:(rt + 1) * 64],
                                                  tpa[0:64, 64:128])
                            nc.vector.tensor_copy(lg2[0:8, rt * 32:(rt + 1) * 32],
                                                  tpb[0:8, 0:64:2])
                            nc.vector.tensor_copy(st8odd[:, rt * 32:(rt + 1) * 32],
                                                  tpb[0:8, 1:64:2])
                            nc.vector.tensor_copy(stBe[:, rt * 32:(rt + 1) * 32],
                                                  tpb[0:8, 64:128:2])
                            nc.vector.tensor_copy(stBo[:, rt * 32:(rt + 1) * 32],
                                                  tpb[0:8, 65:128:2])
                            return
                        nc.vector.tensor_copy(lg1[:, rt * 128:(rt + 1) * 128],
                                              tpa[0:CH, :])
                        # baseline 8-slot parity layout for slots 16-19
                        nc.vector.tensor_copy(lg2[0:8, rt * 64:(rt + 1) * 64],
                                              tpb[0:8, 0::2])
                        nc.vector.tensor_copy(st8odd[:, rt * 64:(rt + 1) * 64],
                                              tpb[0:8, 1::2])

                    def repl_lists(hh):
                        # replicate/stage the lists for row tiles of half hh:
                        # emitting the first batch mid-topk lets the gathers
                        # start while the DVE is still on top-k of tiles 8-15
                        c1 = slice(hh * 8 * PR, (hh + 1) * 8 * PR)
                        c2 = slice(hh * 8 * (PR // 2), (hh + 1) * 8 * (PR // 2))
                        nc.sync.dma_start(lg2[8:16, c2], st8odd[:, c2])
                        if SPLIT:
                            nc.sync.dma_start(lg1[64:128, c1], stB1[:, c1])
                            nc.sync.dma_start(lg2[64:72, c2], stBe[:, c2])
                            nc.sync.dma_start(lg2[72:80, c2], stBo[:, c2])
                            for g in (1, 2, 3):
                                nc.sync.dma_start(lg2[g * 16:(g + 1) * 16, c2],
                                                  lg2[0:16, c2])
                                nc.sync.dma_start(
                                    lg2[64 + g * 16:64 + (g + 1) * 16, c2],
                                    lg2[64:80, c2])
                        else:
                            for g in range(1, NG):
                                nc.sync.dma_start(lg2[g * 16:(g + 1) * 16, c2],
                                                  lg2[0:16, c2])

                    for it in range(NT + 1):
                        if it - 1 >= 0:
                            emit_lists(it - 1, *tps.pop(it - 1))
                        if it == NT // 2:
                            repl_lists(0)
                        if it < NT:
                            tps[it] = emit_scores_topk(it)
                    repl_lists(1)

                scope_out(_sc)
                _sc = scope_in("gath")
                # ---- phase 2: gathers + reduces + BN moment accum ----
                # one g1 (16 slots) + one g2 (8-slot parity, 4 valid) gather
                # per row tile: gather cost scales with index count, so big
                # batches amortize the Q7 per-request overhead.
                mm = pp.tile([128, OT * N], dt.float32, tag="mm")
                s_sb = pp.tile([128, OT * N], dt.float32, tag="s_sb")
                qac = pp.tile([128, OT * 32], dt.float32, tag="qac")
                sqj = pp.tile([128, 2048], dt.float32, tag="sqj")
                if SPLIT:
                    msplit = pp.tile([128, NT * 64], dt.float32, tag="msplit")
                    ssplit = pp.tile([128, NT * 64], dt.float32, tag="ssplit")
                # pools opened at layer top: a pool boundary here would fence
                # the gathers behind all of phase 1
                if True:
                  if SPLIT:
                    w_ = 64
                    for r2 in range(NT // 2):
                        gA = pg.tile([128, 2048], dt.float32, tag="g1")
                        nc.gpsimd.ap_gather(
                            gA, up[:, 0:N], lg1[:, r2 * 128:(r2 + 1) * 128],
                            channels=128, num_elems=N, d=1, num_idxs=2048)
                        gB = pg.tile([128, 1024], dt.float32, tag="g2")
                        nc.gpsimd.ap_gather(
                            gB, up[:, 0:N], lg2[:, r2 * 64:(r2 + 1) * 64],
                            channels=128, num_elems=N, d=1, num_idxs=1024)
                        msl = slice(r2 * 128, (r2 + 1) * 128)
                        nc.vector.tensor_reduce(
                            out=msplit[:, msl],
                            in_=gA.rearrange("p (n k) -> p n k", k=16),
                            op=Alu.max, axis=mybir.AxisListType.X)
                        m2 = pgs.tile([128, 128], dt.float32, tag="m2")
                        nc.vector.tensor_reduce(
                            out=m2,
                            in_=gB.rearrange("p (n k) -> p n k", k=8)[:, :, 0:4],
                            op=Alu.max, axis=mybir.AxisListType.X)
                        nc.vector.tensor_tensor(out=msplit[:, msl],
                                                in0=msplit[:, msl],
                                                in1=m2, op=Alu.max)
                        s1 = pgs.tile([128, 128], dt.float32, tag="s1")
                        nc.vector.tensor_reduce(
                            out=s1,
                            in_=gA.rearrange("p (n k) -> p n k", k=16),
                            op=Alu.add, axis=mybir.AxisListType.X)
                        s2 = pgs.tile([128, 128], dt.float32, tag="s2")
                        nc.vector.tensor_reduce(
                            out=s2,
                            in_=gB.rearrange("p (n k) -> p n k", k=8)[:, :, 0:4],
                            op=Alu.add, axis=mybir.AxisListType.X)
                        nc.vector.tensor_tensor(out=ssplit[:, msl],
                                                in0=s1, in1=s2, op=Alu.add)
                        nc.scalar.activation(
                            sqj, gA, Act.Square,
                            accum_out=qac[:, 2 * r2:2 * r2 + 1])
                        g2v = gB.rearrange("p (n k) -> p n k", k=8)[:, :, 0:4]
                        sqv = sqj[:, 0:512].rearrange("p (n k) -> p n k", k=4)
                        nc.scalar.activation(
                            sqv, g2v, Act.Square,
                            accum_out=qac[:, 2 * r2 + 1:2 * r2 + 2])
                  else:
                    for ot in range(OT):
                        w_ = ow[ot]
                        wch = ((w_ + 15) // 16) * 16
                        usrc = up[0:wch, ot * N:(ot + 1) * N]
                        for rt in range(NT):
                            msl = slice(ot * N + rt * 128, ot * N + (rt + 1) * 128)
                            g1 = pg.tile([CH, 2048], dt.float32, tag="g1")
                            nc.gpsimd.ap_gather(
                                g1[0:wch, :], usrc,
                                lg1[0:wch, rt * 128: (rt + 1) * 128],
                                channels=wch, num_elems=N, d=1, num_idxs=2048)
                            g2 = pg.tile([CH, 1024], dt.float32, tag="g2")
                            nc.gpsimd.ap_gather(
                                g2[0:wch, :], usrc,
                                lg2[0:wch, rt * 64: (rt + 1) * 64],
                                channels=wch, num_elems=N, d=1, num_idxs=1024)
                            nc.vector.tensor_reduce(
                                out=mm[0:w_, msl],
                                in_=g1[0:w_, :].rearrange("p (n k) -> p n k", k=16),
                                op=Alu.max, axis=mybir.AxisListType.X)
                            m2 = pgs.tile([128, 128], dt.float32, tag="m2")
                            nc.vector.tensor_reduce(
                                out=m2[0:w_, :],
                                in_=g2[0:w_, :].rearrange("p (n k) -> p n k", k=8)[:, :, 0:4],
                                op=Alu.max, axis=mybir.AxisListType.X)
                            nc.vector.tensor_tensor(
                                out=mm[0:w_, msl], in0=mm[0:w_, msl],
                                in1=m2[0:w_, :], op=Alu.max)
                            s1 = pgs.tile([128, 128], dt.float32, tag="s1")
                            nc.vector.tensor_reduce(
                                out=s1[0:w_, :],
                                in_=g1[0:w_, :].rearrange("p (n k) -> p n k", k=16),
                                op=Alu.add, axis=mybir.AxisListType.X)
                            s2 = pgs.tile([128, 128], dt.float32, tag="s2")
                            nc.vector.tensor_reduce(
                                out=s2[0:w_, :],
                                in_=g2[0:w_, :].rearrange("p (n k) -> p n k", k=8)[:, :, 0:4],
                                op=Alu.add, axis=mybir.AxisListType.X)
                            nc.vector.tensor_tensor(
                                out=s_sb[0:w_, msl],
                                in0=s1[0:w_, :], in1=s2[0:w_, :], op=Alu.add)
                            # q: sum of u'^2 over the same neighbors (square to
                            # a scratch tile: in-place would WAR-couple the
                            # gathers to the scalar engine)
                            nc.scalar.activation(
                                sqj[0:w_, :], g1[0:w_, :], Act.Square,
                                accum_out=qac[0:w_, ot * 32 + 2 * rt: ot * 32 + 2 * rt + 1])
                            g2v = g2[0:w_, :].rearrange("p (n k) -> p n k", k=8)[:, :, 0:4]
                            sqv = sqj[0:w_, 0:512].rearrange("p (n k) -> p n k", k=4)
                            nc.scalar.activation(
                                sqv, g2v, Act.Square,
                                accum_out=qac[0:w_, ot * 32 + 2 * rt + 1: ot * 32 + 2 * rt + 2])
                if SPLIT:
                    # fold the upper-half results (points 64-127 of each row
                    # tile, computed at partitions 64-127) back down
                    nc.sync.dma_start(
                        mm[0:64, :].rearrange("p (r n) -> p r n", n=128)[:, :, 0:64],
                        msplit[0:64, :].rearrange("p (r n) -> p r n", n=64))
                    nc.sync.dma_start(
                        mm[0:64, :].rearrange("p (r n) -> p r n", n=128)[:, :, 64:128],
                        msplit[64:128, :].rearrange("p (r n) -> p r n", n=64))
                    nc.sync.dma_start(
                        s_sb[0:64, :].rearrange("p (r n) -> p r n", n=128)[:, :, 0:64],
                        ssplit[0:64, :].rearrange("p (r n) -> p r n", n=64))
                    nc.sync.dma_start(
                        s_sb[0:64, :].rearrange("p (r n) -> p r n", n=128)[:, :, 64:128],
                        ssplit[64:128, :].rearrange("p (r n) -> p r n", n=64))
                    nc.sync.dma_start(qac[0:64, 16:32], qac[64:128, 0:16])
                for ot in range(OT):
                    w_ = ow[ot]
                    nc.vector.tensor_reduce(out=sum_q[0:w_, ot:ot + 1],
                                            in_=qac[0:w_, ot * 32:(ot + 1) * 32],
                                            op=Alu.add, axis=mybir.AxisListType.X)

                scope_out(_sc)
                _sc = scope_in("stat")
                # ---- per-core stat terms + AllReduce ----
                for ot in range(OT):
                    w_ = ow[ot]
                    ssl = s_sb[0:w_, ot * N:(ot + 1) * N]
                    vsl = vv[0:w_, ot * N:(ot + 1) * N]
                    nc.vector.tensor_reduce(out=sum_s[0:w_, ot:ot + 1], in_=ssl,
                                            op=Alu.add, axis=mybir.AxisListType.X)
                    # up is fully consumed by the gathers at this point; reuse
                    # its slice as the accumulate-op scratch destination.
                    junk = up[:, ot * N:(ot + 1) * N]
                    nc.vector.scalar_tensor_tensor(out=junk[0:w_, :], in0=ssl, scalar=1.0,
                                                   in1=vsl, op0=Alu.mult, op1=Alu.mult,
                                                   accum_out=svs[0:w_, ot:ot + 1])
                    nc.vector.scalar_tensor_tensor(out=junk[0:w_, :], in0=vsl, scalar=1.0,
                                                   in1=vsl, op0=Alu.mult, op1=Alu.mult,
                                                   accum_out=sv2[0:w_, ot:ot + 1])
                    nc.scalar.mul(vbar[0:w_, ot:ot + 1], sum_v[0:w_, ot:ot + 1], 1.0 / N)
                    nc.vector.scalar_tensor_tensor(out=beta[0:w_, ot:ot + 1],
                                                   in0=neg_mu[0:w_, ot:ot + 1], scalar=-1.0,
                                                   in1=vbar[0:w_, ot:ot + 1],
                                                   op0=Alu.mult, op1=Alu.add)
                    nc.vector.scalar_tensor_tensor(out=t1a[0:w_, ot:ot + 1],
                                                   in0=vbar[0:w_, ot:ot + 1], scalar=-float(N),
                                                   in1=sum_v[0:w_, ot:ot + 1],
                                                   op0=Alu.mult, op1=Alu.add)
                    nc.vector.scalar_tensor_tensor(out=S1p[0:w_, ot:ot + 1],
                                                   in0=t1a[0:w_, ot:ot + 1], scalar=float(K),
                                                   in1=sum_s[0:w_, ot:ot + 1],
                                                   op0=Alu.mult, op1=Alu.add)
                    nc.vector.tensor_tensor(out=tA[0:w_, ot:ot + 1], in0=vbar[0:w_, ot:ot + 1],
                                            in1=sum_s[0:w_, ot:ot + 1], op=Alu.mult)
                    nc.vector.scalar_tensor_tensor(out=tA[0:w_, ot:ot + 1],
                                                   in0=tA[0:w_, ot:ot + 1], scalar=-1.0,
                                                   in1=svs[0:w_, ot:ot + 1],
                                                   op0=Alu.mult, op1=Alu.add)
                    nc.vector.tensor_tensor(out=tBt[0:w_, ot:ot + 1], in0=vbar[0:w_, ot:ot + 1],
                                            in1=vbar[0:w_, ot:ot + 1], op=Alu.mult)
                    nc.vector.scalar_tensor_tensor(out=tBt[0:w_, ot:ot + 1],
                                                   in0=tBt[0:w_, ot:ot + 1], scalar=-float(N),
                                                   in1=sv2[0:w_, ot:ot + 1],
                                                   op0=Alu.mult, op1=Alu.add)
                    nc.vector.scalar_tensor_tensor(out=S2p[0:w_, ot:ot + 1],
                                                   in0=tA[0:w_, ot:ot + 1], scalar=2.0,
                                                   in1=sum_q[0:w_, ot:ot + 1],
                                                   op0=Alu.mult, op1=Alu.add)
                    nc.vector.scalar_tensor_tensor(out=S2p[0:w_, ot:ot + 1],
                                                   in0=tBt[0:w_, ot:ot + 1], scalar=float(K),
                                                   in1=S2p[0:w_, ot:ot + 1],
                                                   op0=Alu.mult, op1=Alu.add)
                    cntl = float(N * K)
                    nc.vector.scalar_tensor_tensor(out=ar_in[0:w_, 2 * ot:2 * ot + 1],
                                                   in0=beta[0:w_, ot:ot + 1], scalar=cntl,
                                                   in1=S1p[0:w_, ot:ot + 1],
                                                   op0=Alu.mult, op1=Alu.add)
                    nc.vector.tensor_tensor(out=tC[0:w_, ot:ot + 1], in0=beta[0:w_, ot:ot + 1],
                                            in1=S1p[0:w_, ot:ot + 1], op=Alu.mult)
                    nc.vector.scalar_tensor_tensor(out=tC[0:w_, ot:ot + 1],
                                                   in0=tC[0:w_, ot:ot + 1], scalar=2.0,
                                                   in1=S2p[0:w_, ot:ot + 1],
                                                   op0=Alu.mult, op1=Alu.add)
                    nc.vector.tensor_tensor(out=tD[0:w_, ot:ot + 1], in0=beta[0:w_, ot:ot + 1],
                                            in1=beta[0:w_, ot:ot + 1], op=Alu.mult)
                    nc.vector.scalar_tensor_tensor(out=ar_in[0:w_, 2 * ot + 1:2 * ot + 2],
                                                   in0=tD[0:w_, ot:ot + 1], scalar=cntl,
                                                   in1=tC[0:w_, ot:ot + 1],
                                                   op0=Alu.mult, op1=Alu.add)

                with tc.tile_pool(name=f"dr{li}", bufs=1, space="DRAM") as dram:
                    ari = dram.tile([128, 2 * OT], dt.float32)
                    aro = dram.tile([128, 2 * OT], dt.float32)
                    nc.sync.dma_start(ari[:], ar_in[:, 0:2 * OT])
                    nc.gpsimd.collective_compute(
                        "AllReduce", Alu.add, replica_groups=[list(range(8))],
                        ins=[ari.opt()], outs=[aro.opt()])
                    nc.sync.dma_start(ar_out[:, 0:2 * OT], aro[:])
                # overlap with the AllReduce: mm <- m' + v (no AR dependency)
                for ot in range(OT):
                    w_ = ow[ot]
                    nc.vector.tensor_tensor(out=mm[0:w_, ot * N:(ot + 1) * N],
                                            in0=mm[0:w_, ot * N:(ot + 1) * N],
                                            in1=vv[0:w_, ot * N:(ot + 1) * N], op=Alu.add)

                # post-AR: mean/var/scale/bias + activation
                nc.sync.dma_start(gsb[0:ow[0], 0:1], gv[li][0:ow[0], :])
                nc.sync.dma_start(bsb[0:ow[0], 0:1], bv[li][0:ow[0], :])
                if OT > 1:
                    nc.sync.dma_start(gsb[0:ow[1], 1:2], gv[li][128:128 + ow[1], :])
                    nc.sync.dma_start(bsb[0:ow[1], 1:2], bv[li][128:128 + ow[1], :])
                cntg = float(B * N * K)
                for ot in range(OT):
                    w_ = ow[ot]
                    nc.scalar.mul(mean[0:w_, ot:ot + 1], ar_out[0:w_, 2 * ot:2 * ot + 1], 1.0 / cntg)
                    nc.scalar.mul(e2[0:w_, ot:ot + 1], ar_out[0:w_, 2 * ot + 1:2 * ot + 2], 1.0 / cntg)
                    nc.vector.tensor_tensor(out=varp[0:w_, ot:ot + 1], in0=mean[0:w_, ot:ot + 1],
                                            in1=mean[0:w_, ot:ot + 1], op=Alu.mult)
                    nc.vector.scalar_tensor_tensor(out=varp[0:w_, ot:ot + 1],
                                                   in0=varp[0:w_, ot:ot + 1], scalar=-1.0,
                                                   in1=e2[0:w_, ot:ot + 1],
                                                   op0=Alu.mult, op1=Alu.add)
                    nc.vector.tensor_scalar_add(varp[0:w_, ot:ot + 1], varp[0:w_, ot:ot + 1], EPS)
                    nc.vector.reciprocal(rec[0:w_, ot:ot + 1], varp[0:w_, ot:ot + 1])
                    nc.scalar.sqrt(rsq[0:w_, ot:ot + 1], rec[0:w_, ot:ot + 1])
                    nc.vector.tensor_tensor(out=aco[0:w_, ot:ot + 1], in0=gsb[0:w_, ot:ot + 1],
                                            in1=rsq[0:w_, ot:ot + 1], op=Alu.mult)
                    nc.vector.tensor_tensor(out=bi[0:w_, ot:ot + 1], in0=neg_mu[0:w_, ot:ot + 1],
                                            in1=mean[0:w_, ot:ot + 1], op=Alu.add)
                    nc.vector.tensor_tensor(out=bi[0:w_, ot:ot + 1], in0=bi[0:w_, ot:ot + 1],
                                            in1=aco[0:w_, ot:ot + 1], op=Alu.mult)
                    nc.vector.scalar_tensor_tensor(out=bi[0:w_, ot:ot + 1],
                                                   in0=bi[0:w_, ot:ot + 1], scalar=-1.0,
                                                   in1=bsb[0:w_, ot:ot + 1],
                                                   op0=Alu.mult, op1=Alu.add)
                    if li == 0:
                        dst = h[0][0:64, :]
                    elif li == 1:
                        dst = x2_sb[:, :]
                    elif li == 2:
                        dst = h[1][:, :]
                    else:
                        dst = h[2][:, :] if ot == 0 else h[3][:, :]
                    zt = up[:, ot * N:(ot + 1) * N]
                    nc.scalar.activation(zt[0:w_, :], mm[0:w_, ot * N:(ot + 1) * N],
                                         Act.Identity,
                                         bias=bi[0:w_, ot:ot + 1], scale=aco[0:w_, ot:ot + 1])
                    nc.vector.scalar_tensor_tensor(out=dst[0:w_, :],
                                                   in0=zt[0:w_, :], scalar=0.2,
                                                   in1=zt[0:w_, :], op0=Alu.mult, op1=Alu.max)
                    if li == 1:
                        nc.sync.dma_start(h[0][64:128, :], x2_sb)
                scope_out(_sc)

        # ---------------- final conv + BN + lrelu ----------------
        with tc.tile_pool(name="pf", bufs=1) as pf, \
             tc.tile_pool(name="pfp", bufs=2, space="PSUM") as pfp:
            y_sb = []
            for ob in range(8):
                ytile = pf.tile([128, N], dt.float32, tag=f"y{ob}")
                y_sb.append(ytile)
            sm5 = pf.tile([128, 160], dt.float32, tag="sm5")
            sum_y = sm5[:, 0:8]
            mu5 = sm5[:, 8:16]
            nmu5 = sm5[:, 16:24]
            syc2 = sm5[:, 24:32]
            tE = sm5[:, 32:40]
            tF = sm5[:, 40:48]
            g5_sb = sm5[:, 48:56]
            b5_sb = sm5[:, 56:64]
            mean5 = sm5[:, 64:72]
            e25 = sm5[:, 72:80]
            var5 = sm5[:, 80:88]
            rec5 = sm5[:, 88:96]
            rsq5 = sm5[:, 96:104]
            a5 = sm5[:, 104:112]
            c5 = sm5[:, 112:120]
            ar5_in = sm5[:, 120:136]
            ar5_out = sm5[:, 136:152]
            w5T_sb = pf.tile([128, 4 * 1024], dt.float32, tag="w5T_sb")
            for cb in range(4):
                nc.sync.dma_start(w5T_sb[:, cb * 1024:(cb + 1) * 1024],
                                  w5T_d[cb * 128:(cb + 1) * 128, :])
            w5b = pf.tile([128, 4 * 1024], dt.bfloat16, tag="w5b")
            nc.vector.tensor_copy(w5b, w5T_sb)
            hb = []
            for cb in range(4):
                hbt = pf.tile([128, N], dt.bfloat16, tag=f"hb{cb}")
                nc.vector.tensor_copy(hbt, h[cb])
                hb.append(hbt)
            for ob in range(8):
                for nch in range(NCH):
                    yp = pfp.tile([128, 512], dt.float32, tag="yp")
                    for cb in range(4):
                        nc.tensor.matmul(yp, lhsT=w5b[:, cb * 1024 + ob * 128:
                                                      cb * 1024 + (ob + 1) * 128],
                                         rhs=hb[cb][:, nch * 512:(nch + 1) * 512],
                                         start=(cb == 0), stop=(cb == 3))
                    nc.scalar.activation(y_sb[ob][:, nch * 512:(nch + 1) * 512], yp, Act.Copy)
                nc.vector.tensor_reduce(out=sum_y[:, ob:ob + 1], in_=y_sb[ob],
                                        op=Alu.add, axis=mybir.AxisListType.X)
                nc.scalar.mul(mu5[:, ob:ob + 1], sum_y[:, ob:ob + 1], 1.0 / N)
                nc.scalar.mul(nmu5[:, ob:ob + 1], sum_y[:, ob:ob + 1], -1.0 / N)
                yc = pf.tile([128, N], dt.float32, tag="yc")
                nc.scalar.activation(yc, y_sb[ob], Act.Identity,
                                     bias=nmu5[:, ob:ob + 1], scale=1.0)
                junk5 = pf.tile([128, N], dt.float32, tag="junk5")
                nc.vector.scalar_tensor_tensor(out=junk5, in0=yc, scalar=1.0, in1=yc,
                                               op0=Alu.mult, op1=Alu.mult,
                                               accum_out=syc2[:, ob:ob + 1])
                nc.vector.tensor_copy(ar5_in[:, 2 * ob:2 * ob + 1], sum_y[:, ob:ob + 1])
                nc.vector.tensor_tensor(out=tE[:, ob:ob + 1], in0=mu5[:, ob:ob + 1],
                                        in1=sum_y[:, ob:ob + 1], op=Alu.mult)
                nc.vector.scalar_tensor_tensor(out=tE[:, ob:ob + 1], in0=tE[:, ob:ob + 1],
                                               scalar=2.0, in1=syc2[:, ob:ob + 1],
                                               op0=Alu.mult, op1=Alu.add)
                nc.vector.tensor_tensor(out=tF[:, ob:ob + 1], in0=mu5[:, ob:ob + 1],
                                        in1=mu5[:, ob:ob + 1], op=Alu.mult)
                nc.vector.scalar_tensor_tensor(out=ar5_in[:, 2 * ob + 1:2 * ob + 2],
                                               in0=tF[:, ob:ob + 1], scalar=-float(N),
                                               in1=tE[:, ob:ob + 1],
                                               op0=Alu.mult, op1=Alu.add)
            with tc.tile_pool(name="dr5", bufs=1, space="DRAM") as dram5:
                ari5 = dram5.tile([128, 16], dt.float32)
                aro5 = dram5.tile([128, 16], dt.float32)
                nc.sync.dma_start(ari5[:], ar5_in)
                nc.gpsimd.collective_compute(
                    "AllReduce", Alu.add, replica_groups=[list(range(8))],
                    ins=[ari5.opt()], outs=[aro5.opt()])
                nc.sync.dma_start(ar5_out, aro5[:])
            for ob in range(8):
                nc.sync.dma_start(g5_sb[:, ob:ob + 1], g5_d[ob * 128:(ob + 1) * 128, :])
                nc.sync.dma_start(b5_sb[:, ob:ob + 1], b5_d[ob * 128:(ob + 1) * 128, :])
            cnt5 = float(B * N)
            for ob in range(8):
                nc.scalar.mul(mean5[:, ob:ob + 1], ar5_out[:, 2 * ob:2 * ob + 1], 1.0 / cnt5)
                nc.scalar.mul(e25[:, ob:ob + 1], ar5_out[:, 2 * ob + 1:2 * ob + 2], 1.0 / cnt5)
                nc.vector.tensor_tensor(out=var5[:, ob:ob + 1], in0=mean5[:, ob:ob + 1],
                                        in1=mean5[:, ob:ob + 1], op=Alu.mult)
                nc.vector.scalar_tensor_tensor(out=var5[:, ob:ob + 1], in0=var5[:, ob:ob + 1],
                                               scalar=-1.0, in1=e25[:, ob:ob + 1],
                                               op0=Alu.mult, op1=Alu.add)
                nc.vector.tensor_scalar_add(var5[:, ob:ob + 1], var5[:, ob:ob + 1], EPS)
                nc.vector.reciprocal(rec5[:, ob:ob + 1], var5[:, ob:ob + 1])
                nc.scalar.sqrt(rsq5[:, ob:ob + 1], rec5[:, ob:ob + 1])
                nc.vector.tensor_tensor(out=a5[:, ob:ob + 1], in0=g5_sb[:, ob:ob + 1],
                                        in1=rsq5[:, ob:ob + 1], op=Alu.mult)
                nc.vector.tensor_tensor(out=c5[:, ob:ob + 1], in0=mean5[:, ob:ob + 1],
                                        in1=a5[:, ob:ob + 1], op=Alu.mult)
                nc.vector.scalar_tensor_tensor(out=c5[:, ob:ob + 1], in0=c5[:, ob:ob + 1],
                                               scalar=-1.0, in1=b5_sb[:, ob:ob + 1],
                                               op0=Alu.mult, op1=Alu.add)
                z5 = pf.tile([128, N], dt.float32, tag="z5")
                nc.scalar.activation(z5, y_sb[ob], Act.Identity,
                                     bias=c5[:, ob:ob + 1], scale=a5[:, ob:ob + 1])
                o5 = pf.tile([128, N], dt.float32, tag="o5")
                nc.vector.scalar_tensor_tensor(out=o5, in0=z5, scalar=0.2, in1=z5,
                                               op0=Alu.mult, op1=Alu.max)
                nc.sync.dma_start(out_d[ob * 128:(ob + 1) * 128, :], o5)
            for i in range(4):
                nc.sync.dma_start(xdbg[i], h[i])

    nc.compile()
    return nc


def _get_compiled():
    if "nc" not in _CACHE:
        _CACHE["nc"] = _build()
    return _CACHE["nc"]


def _make_in_maps(inputs):
    x = np.ascontiguousarray(np.asarray(inputs["x"], dtype=np.float32))
    shared = {}
    Cs = [3, 64, 64, 128]
    for li in range(4):
        w = np.asarray(inputs[f"w{li+1}"], dtype=np.float32)
        C = Cs[li]
        shared[f"waT{li}"] = np.ascontiguousarray(w[:, :C].T)
        shared[f"wbmaT{li}"] = np.ascontiguousarray((w[:, C:] - w[:, :C]).T)
        shared[f"g{li}"] = np.ascontiguousarray(
            np.asarray(inputs[f"g{li+1}"], np.float32).reshape(-1, 1))
        shared[f"b{li}"] = np.ascontiguousarray(
            np.asarray(inputs[f"b{li+1}"], np.float32).reshape(-1, 1))
    shared["w5T"] = np.ascontiguousarray(np.asarray(inputs["w5"], np.float32).T)
    shared["g5"] = np.ascontiguousarray(np.asarray(inputs["g5"], np.float32).reshape(-1, 1))
    shared["b5"] = np.ascontiguousarray(np.asarray(inputs["b5"], np.float32).reshape(-1, 1))

    return [dict(shared, x0=np.ascontiguousarray(x[i])) for i in range(B)]


def kernel(**inputs):
    from concourse.bass_utils import run_bass_kernel_spmd

    nc = _get_compiled()
    in_maps = _make_in_maps(inputs)
    res = run_bass_kernel_spmd(nc, in_maps, core_ids=list(range(8)))
    out = np.stack([res.results[i]["out"] for i in range(B)]).astype(np.float32)
    return out
